# revision 39
# baseline (speedup 1.0000x reference)
"""GAT layer (PyG GATConv H=4,C=64 + PReLU) on 8 Trainium2 NeuronCores.

Strategy (graph/data parallel, dst-sharded):
  - Host: add self loops, sort edges by dst, partition dst-nodes across 8
    cores (6250 each), group each core's edges into 128-dst "blocks", pair
    blocks into GROUPS, and tile each (group, window-half)'s edges into
    128-edge tiles.  Tile/chunk structure is uniform across cores so ONE
    SPMD program serves all 8 cores; per-core divergence rides in data.
  - Node rotation: core m's table stores node (m*6250 + l) mod N at row
    l+1, fed by a host-rotated copy of x.  Hence every core's OWN dst
    nodes are rows 1..6250 — the per-edge a_dst gather uses one small
    int16-indexed window identical on all cores.
  - Phase 1 (per core, replicated matmul): table row = [a_dst(4) | h(256)
    | a_src(4) | junk] (bf16, 768B stride).  h = x @ lin_w.T on PE;
    a_src/a_dst fold into the same matmul as extra columns (w_a =
    lin_w.T @ att).  Rows 0 and N+1 are sentinels with a_src=-30000 so
    padded edges get p = exp(lrelu(-big)) = 0.  Table writes are staged
    in SBUF per node-chunk and issued as ONE batched DMA per chunk on
    the scalar (ACT) HWDGE queue, overlapping the SP queue's x loads.
    PSUM->staging copies alternate between ACT and DVE.
  - Phase 2: per (block, half) chunk (up to tc_max tiles in one
    dma_gather; SWDGE ring enlarged via dynamic_dma_scratch_size),
    gather table rows by src, build one-hot S1[k,slot] by iota-compare,
    p = exp(leaky_relu(a_src+a_dst)) (lrelu on DVE via max(0.2e,e), exp
    on ACT written straight into the gathered row), messages h*p
    scatter-added into each block's 128 dst slots by matmul
    (lhsT=S1, rhs=[h*p | p]) accumulating [msg | denom] in PSUM.
    Epilogue per block: out = prelu(msg/denom + bias).
  - Per-edge a_dst needs NO DMA gather (the 256B-elem per-edge gather
    dominated the old kernel: sub-512B SDMA descriptors run at half
    rate).  Instead: a_dst of the core's own 6250 dst slots is stashed
    from phase-1 PSUM ([128, nblk, 4] in SBUF); per chunk, rel (slot id
    per edge) is broadcast to all partitions by a PE outer product
    ones^T @ rel_row (rel_row streamed to partition 0 per block), a
    transposed one-hot S1T[slot,e]=(slot==rel[e]) is built by DVE
    iota-compare, and a tiny PE matmul S1T^T @ adst_own[:,b,:] yields
    per-edge a_dst in PSUM, read directly by the DVE add.
  - All per-core inputs ride in ONE packed int16 tensor (midx | relT |
    rel | w | bias | prelu | sentinels | x pre-rearranged), bitcast
    per region on the DRAM side; 2 buffers per core total keeps the
    per-dispatch host cost down.
  - Softmax max-subtraction is skipped (logits are O(1); exp can't
    overflow) making the edge pass single-sweep: out = (Σ p·h)/(Σ p).
"""

import sys

sys.path.insert(0, "/opt/trn_rl_repo")

import numpy as np
import ml_dtypes

import concourse.bass as bass
import concourse.bacc as bacc
import concourse.tile as tile
from concourse import mybir
from concourse.bass import AP

F32 = mybir.dt.float32
BF16 = mybir.dt.bfloat16
I16 = mybir.dt.int16
AF = mybir.ActivationFunctionType
OP = mybir.AluOpType
BF16NP = ml_dtypes.bfloat16

P = 128
NEG_SLOPE = 0.2
SENT_NEG = -30000.0


class Cfg:
    def __init__(self, n_nodes=50000, in_ch=512, hid=64, heads=4, n_cores=8,
                 tc_max=8, node_chunk=2048, win=32768, group=1,
                 dma_scratch=49152):
        assert n_nodes % n_cores == 0
        assert in_ch % P == 0
        self.n_nodes = n_nodes
        self.in_ch = in_ch
        self.hid = hid
        self.heads = heads
        self.hc = hid * heads                      # 256
        self.rowp = self.hc + 2 * heads            # 264 payload cols
        self.row = 384                             # table row stride (768B)
        self.gcol = self.row                       # gather full 768B rows
        self.n_cores = n_cores
        self.ndst = n_nodes // n_cores             # 6250
        self.nblk = -(-self.ndst // P)             # 49
        self.kt = in_ch // P                       # 4
        self.tc_max = tc_max
        self.node_chunk = node_chunk
        self.rows = n_nodes + 2                    # + two sentinel rows
        self.win = min(win, self.rows)             # int16 gather window
        assert self.rows <= 2 * self.win, "lo+hi windows must cover table"
        self.group = group
        self.dma_scratch = dma_scratch
        self.ring = dma_scratch // 16              # SWDGE descriptor ring
        self.n_swq = 1                             # SWDGE queues (gathers)


CFG = Cfg()


def _wrap16(flat):
    """int16 index list -> dma_gather layout [128, n/16] (i -> [i%16, i//16],
    replicated to all 8 Q7 core groups)."""
    n = len(flat)
    assert n % 16 == 0
    a = np.asarray(flat, dtype=np.int16).reshape(n // 16, 16).T  # [16, n/16]
    return np.tile(a, (8, 1))                                    # [128, 128]


# ---------------------------------------------------------------- host prep

def host_prep_edges(edge_index, cfg):
    """Sort/partition/tile edges.  Returns (per_core data dicts, meta).

    Tile column space: for each group of `cfg.group` blocks, first all LO
    tiles (block-major), then all HI tiles.  Chunks (= one dma_gather each)
    split each (group, half) span at tc_max tiles.
    """
    n = cfg.n_nodes
    loop = np.arange(n, dtype=np.int64)
    src = np.concatenate([edge_index[0].astype(np.int64), loop])
    dst = np.concatenate([edge_index[1].astype(np.int64), loop])
    order = np.argsort(dst, kind="stable")
    src_s = src[order]
    dst_s = dst[order]

    lo_rows = cfg.win                 # lo window = rows [0, win)
    hi_base = cfg.rows - cfg.win      # hi window = rows [hi_base, rows)
    sent_hi_rel = cfg.rows - 1 - hi_base

    # per-(core, block) segments; rotated src rows; lo/hi split
    seg = {}
    tl_req = np.zeros((cfg.n_cores, cfg.nblk), dtype=np.int64)
    th_req = np.zeros((cfg.n_cores, cfg.nblk), dtype=np.int64)
    for m in range(cfg.n_cores):
        base = m * cfg.ndst
        for b in range(cfg.nblk):
            d0 = base + b * P
            d1 = min(base + (b + 1) * P, base + cfg.ndst)
            lo = np.searchsorted(dst_s, d0)
            hi = np.searchsorted(dst_s, d1)
            s_rot = (src_s[lo:hi] - base) % n + 1     # rotated table row
            d_loc = dst_s[lo:hi] - d0                 # slot in block
            a_idx = dst_s[lo:hi] - base + 1           # rotated a_dst row
            is_lo = s_rot < lo_rows
            seg[(m, b)] = (s_rot, d_loc, a_idx, is_lo)
            tl_req[m, b] = -(-int(is_lo.sum()) // P)
            th_req[m, b] = -(-int((~is_lo).sum()) // P)
    tl = tl_req.max(axis=0).astype(int)
    th = th_req.max(axis=0).astype(int)
    for b in range(cfg.nblk):
        if tl[b] + th[b] == 0:
            tl[b] = 1

    # ---- tile column layout: group-major, half-major, block-major
    ngrp = -(-cfg.nblk // cfg.group)
    groups = [list(range(g * cfg.group, min((g + 1) * cfg.group, cfg.nblk)))
              for g in range(ngrp)]
    col0 = {}            # (b, half) -> first tile column
    blk_span = {}        # b -> (t_first, t_last) inclusive tile cols
    chunks = []          # dicts: b-range via tiles list, t0, nt, icol, half
    icol = 0
    T = 0
    for blks in groups:
        for half in ("lo", "hi"):
            cnt = tl if half == "lo" else th
            span_t0 = T
            for b in blks:
                col0[(b, half)] = T
                T += int(cnt[b])
            span_nt = T - span_t0
            # split the span into gather chunks of <= tc_max tiles
            q0 = 0
            while q0 < span_nt:
                qq = min(cfg.tc_max, span_nt - q0)
                assert qq * P <= cfg.ring, (qq, cfg.ring)
                chunks.append(dict(t0=span_t0 + q0, nt=qq, nidx=qq * P,
                                   icol=icol, half=half))
                icol += qq * P // 16
                q0 += qq
    icol_main = icol
    for b in range(cfg.nblk):
        ts, te = [], []
        for half, cnt in (("lo", tl), ("hi", th)):
            if cnt[b]:
                ts.append(col0[(b, half)])
                te.append(col0[(b, half)] + int(cnt[b]) - 1)
        blk_span[b] = (min(ts), max(te))

    # relT layout: partition b holds block b's rel values (lo tiles then hi
    # tiles, matching the global tile-column order restricted to b)
    assert cfg.nblk <= P
    rtc = int((tl + th).max()) * P

    per_core = []
    for m in range(cfg.n_cores):
        rel_all = np.zeros((P, T), dtype=np.float32)
        relt = np.zeros((P, rtc), dtype=np.float32)
        midx_all = np.zeros((P, icol_main), dtype=np.int16)
        sidx_all = np.zeros((P, T * P // 16), dtype=np.int16)  # per-tile src
        for b in range(cfg.nblk):
            s_rot, d_loc, a_idx, is_lo = seg[(m, b)]
            boff = 0
            for half, cnt in (("lo", tl), ("hi", th)):
                nt_half = int(cnt[b])
                if nt_half == 0:
                    continue
                sel = is_lo if half == "lo" else ~is_lo
                ne = int(sel.sum())
                npad = nt_half * P
                if half == "lo":
                    bs = np.zeros(npad, dtype=np.int64)       # sentinel_lo
                else:
                    bs = np.full(npad, sent_hi_rel + hi_base, dtype=np.int64)
                br = np.zeros(npad, dtype=np.int64)
                bs[:ne] = s_rot[sel]
                br[:ne] = d_loc[sel]
                if half == "hi":
                    bs -= hi_base
                assert bs.min() >= 0 and bs.max() < cfg.win
                t0 = col0[(b, half)]
                rel_all[:, t0:t0 + nt_half] = br.reshape(nt_half, P).T
                relt[b, boff:boff + npad] = br
                boff += npad
                sidx_all[:, t0 * P // 16:(t0 + nt_half) * P // 16] = \
                    _wrap16(bs)
        for ch in chunks:
            c0, c1 = ch["t0"] * P // 16, (ch["t0"] + ch["nt"]) * P // 16
            midx_all[:, ch["icol"]:ch["icol"] + ch["nidx"] // 16] = \
                sidx_all[:, c0:c1]
        rel2 = np.repeat(rel_all.astype(BF16NP), 2, axis=1)   # [P, 2T]
        per_core.append(dict(
            midx=np.ascontiguousarray(midx_all),
            relt=np.ascontiguousarray(relt.astype(BF16NP)),
            rel=np.ascontiguousarray(rel2),
        ))
    meta = dict(chunks=chunks, tl=tl, th=th, col0=col0, blk_span=blk_span,
                groups=groups, T=T, icol_main=icol_main, rtc=rtc,
                hi_base=hi_base)
    meta["off"] = _pack_offsets(cfg, meta)
    return per_core, meta


def _pack_offsets(cfg, meta):
    """Column offsets (int16 units) of each region in the packed input."""
    off = {}
    c = 0
    for name, cols in (
        ("midx", meta["icol_main"]),
        ("relt", meta["rtc"]),
        ("rel", 2 * meta["T"]),
        ("w", cfg.kt * cfg.rowp),
        ("bias", 2 * cfg.hc),
        ("pw", 2 * cfg.hc),
        ("sent", cfg.rowp),
        ("xt", cfg.kt * cfg.n_nodes),
    ):
        off[name] = c
        c += cols
    off["end"] = c
    return off


def pack_inputs(pc, shared, m, cfg, meta):
    """One [128, PKC] int16 tensor holding every per-core input."""
    off = meta["off"]
    pk = np.zeros((P, off["end"]), dtype=np.int16)

    def put(name, arr):
        a = np.ascontiguousarray(arr).view(np.int16)
        pk[:a.shape[0], off[name]:off[name] + a.shape[1]] = a

    put("midx", pc["midx"])
    put("relt", pc["relt"])
    put("rel", pc["rel"])
    # rhs_w pre-transposed to the matmul layout: w[p, k*rowp + r]
    rw = shared["rhs_w"]                      # [in_ch, rowp] bf16
    wt = rw.reshape(cfg.kt, P, cfg.rowp).transpose(1, 0, 2).reshape(
        P, cfg.kt * cfg.rowp)
    put("w", wt)
    put("bias", shared["bias_rep"])
    put("pw", shared["pw_rep"])
    put("sent", shared["sent"])
    # xT pre-rearranged to the xg layout: x[p, k*n + q] = xT[k*128+p, q]
    xr = shared["xTs"][m].reshape(cfg.kt, P, cfg.n_nodes).transpose(
        1, 0, 2).reshape(P, cfg.kt * cfg.n_nodes)
    put("xt", xr)
    return pk


def host_prep_weights(x, lin_w, att_src, att_dst, bias, prelu_w, cfg):
    n, ic, h, c = cfg.n_nodes, cfg.in_ch, cfg.heads, cfg.hid
    w3 = lin_w.astype(np.float64).reshape(h, c, ic)
    wa_src = (w3 * att_src.astype(np.float64).reshape(h, c, 1)).sum(1).T
    wa_dst = (w3 * att_dst.astype(np.float64).reshape(h, c, 1)).sum(1).T
    lwT = lin_w.astype(np.float64).T                           # [ic, 256]
    lwTi = lwT.reshape(ic, h, c).transpose(0, 2, 1).reshape(ic, h * c)
    rhs = np.concatenate([wa_dst, lwTi, wa_src], axis=1)       # [ic, 264]
    rhs_w = np.ascontiguousarray(rhs.astype(BF16NP))
    def inter(v):
        return v.reshape(h, c).T.reshape(h * c)
    bias_rep = np.ascontiguousarray(np.broadcast_to(
        inter(bias.astype(np.float32)), (P, cfg.hc)))
    pw_rep = np.ascontiguousarray(np.broadcast_to(
        inter(prelu_w.astype(np.float32)), (P, cfg.hc)))
    sent = np.zeros((2, cfg.rowp), dtype=BF16NP)
    sent[:, cfg.rowp - cfg.heads:] = SENT_NEG      # a_src cols
    xbf = x.astype(BF16NP)
    xTs = []
    for m in range(cfg.n_cores):
        r = np.roll(xbf, -m * cfg.ndst, axis=0)
        xTs.append(np.ascontiguousarray(r.T))
    return dict(rhs_w=rhs_w, bias_rep=bias_rep, pw_rep=pw_rep, sent=sent,
                xTs=xTs)


# ---------------------------------------------------------------- builder

def build(cfg, meta, bias_nonzero=True, parts=None):
    parts = parts if parts is not None else {
        "p1", "gather", "adst", "s1", "pcomp", "mm", "epi"}
    if "p1" in parts:
        parts |= {"p1x", "p1mm", "p1w"}
    n, row, hc, h = cfg.n_nodes, cfg.row, cfg.hc, cfg.heads
    nc = bacc.Bacc(dynamic_dma_scratch_size=cfg.dma_scratch,
                   num_swdge_queues=cfg.n_swq)

    off = meta["off"]
    pk = nc.declare_dram_parameter("pk", [P, off["end"]], I16,
                                   isOutput=False)
    out = nc.declare_dram_parameter("out", [cfg.ndst, hc], F32, isOutput=True)

    def pkv(name, cols, dt=None, rows=(0, P)):
        v = pk[rows[0]:rows[1], off[name]:off[name] + cols]
        return v.bitcast(dt) if dt is not None else v

    table = nc.dram_tensor("table", [cfg.rows, row], BF16)
    T = meta["T"]
    hi_base = meta["hi_base"]

    with tile.TileContext(nc) as tc:
        fpool_cm = tc.tile_pool(name="p2f", bufs=1)
        fpool = fpool_cm.__enter__()

        midx_sb = fpool.tile([P, meta["icol_main"]], I16)
        nc.sync.dma_start(out=midx_sb[:], in_=pkv("midx", meta["icol_main"]))
        rel_sb = fpool.tile([P, 2 * T], BF16)
        nc.sync.dma_start(out=rel_sb[:], in_=pkv("rel", 2 * T, BF16))
        bias_sb = fpool.tile([P, hc], F32)
        nc.sync.dma_start(out=bias_sb[:], in_=pkv("bias", 2 * hc, F32))
        pw_sb = fpool.tile([P, hc], F32)
        nc.sync.dma_start(out=pw_sb[:], in_=pkv("pw", 2 * hc, F32))

        iota_i = fpool.tile([P, P], mybir.dt.int32)
        nc.gpsimd.iota(iota_i[:], pattern=[[1, P]], base=0,
                       channel_multiplier=0)
        iota_f = fpool.tile([P, P], F32)
        nc.vector.tensor_copy(out=iota_f[:], in_=iota_i[:])
        iota_bf = fpool.tile([P, P], BF16)
        nc.vector.tensor_copy(out=iota_bf[:], in_=iota_f[:])
        # per-partition index column (for the transposed one-hot)
        iotap_i = fpool.tile([P, 1], mybir.dt.int32)
        nc.gpsimd.iota(iotap_i[:], pattern=[[0, 1]], base=0,
                       channel_multiplier=1)
        iotap_f = fpool.tile([P, 1], F32)
        nc.vector.tensor_copy(out=iotap_f[:], in_=iotap_i[:])
        iotap_bf = fpool.tile([P, 1], BF16)
        nc.vector.tensor_copy(out=iotap_bf[:], in_=iotap_f[:])
        # all-ones row for the PE partition-broadcast (ones^T @ rel_row)
        ones_row = fpool.tile([1, P], BF16)
        nc.vector.memset(ones_row[:], 1.0)
        # per-block a_dst of this core's own dst slots, filled in phase 1
        adst_own = fpool.tile([P, cfg.nblk, h], BF16)

        # ---------------- phase 1: build table ----------------
        with (
            tc.tile_pool(name="p1w", bufs=1) as wpool,
            tc.tile_pool(name="p1x", bufs=2) as xpool,
            tc.tile_pool(name="p1o", bufs=2) as opool,
            tc.tile_pool(name="p1ps", bufs=4, space="PSUM") as pspool,
        ):
            w_sb = wpool.tile([P, cfg.kt, cfg.rowp], BF16)
            nc.sync.dma_start(
                out=w_sb[:],
                in_=pkv("w", cfg.kt * cfg.rowp, BF16).rearrange(
                    "p (k r) -> p k r", k=cfg.kt))
            nc.sync.dma_start(out=table[0:1, 0:cfg.rowp],
                              in_=pkv("sent", cfg.rowp, BF16, rows=(0, 1)))
            nc.sync.dma_start(out=table[cfg.rows - 1:cfg.rows, 0:cfg.rowp],
                              in_=pkv("sent", cfg.rowp, BF16, rows=(1, 2)))

            nch = cfg.node_chunk
            p1_any = parts & {"p1x", "p1mm", "p1w"}
            p1_starts = list(range(0, n if p1_any else 0, nch))

            def p1_chunk(n0):
                nn = min(nch, n - n0)
                nt = -(-nn // P)
                xg = xpool.tile([P, cfg.kt, nch], BF16, tag="xg")
                if "p1x" in parts:
                    xtv = pkv("xt", cfg.kt * n, BF16).rearrange(
                        "p (k q) -> p k q", k=cfg.kt)
                    nc.sync.dma_start(
                        out=xg[:, :, :nn], in_=xtv[:, :, n0:n0 + nn])
                stg = opool.tile([P, nch // P, cfg.rowp], BF16, tag="stg")
                for ti in range(nt):
                    t0 = ti * P
                    mm = min(P, nn - t0)
                    if "p1mm" in parts:
                        ps = pspool.tile([P, cfg.rowp], F32, tag="ps")
                        for k in range(cfg.kt):
                            nc.tensor.matmul(
                                ps[:mm, :],
                                lhsT=xg[:, k, t0:t0 + mm],
                                rhs=w_sb[:, k, :],
                                start=(k == 0), stop=(k == cfg.kt - 1))
                        if ti % 2 == 1:
                            nc.vector.tensor_copy(out=stg[:mm, ti, :],
                                                  in_=ps[:mm, :])
                        else:
                            nc.scalar.copy(out=stg[:mm, ti, :],
                                           in_=ps[:mm, :])
                        # own-block a_dst (table rows 1+128b..) align with
                        # the psum tile grid: stash cols 0:4 for phase 2
                        gt = (n0 + t0) // P
                        if gt < cfg.nblk:
                            nc.vector.tensor_copy(
                                out=adst_own[:, gt, :], in_=ps[:, 0:h])
                if "p1w" in parts:
                    # one batched write: DRAM rows [1+n0, 1+n0+nn) cols
                    # 0:rowp at 768B row stride, via the ACT HWDGE queue.
                    # A partial last tile is written separately so garbage
                    # staging rows never land past the table.
                    ft, rem = divmod(nn, P)
                    if ft:
                        tv = table[1 + n0:1 + n0 + ft * P, 0:cfg.rowp]
                        dst = AP(tv.tensor, tv.offset,
                                 [[row, P], [P * row, ft], [1, cfg.rowp]])
                        nc.scalar.dma_start(out=dst, in_=stg[:, :ft, :])
                    if rem:
                        r0 = 1 + n0 + ft * P
                        nc.scalar.dma_start(
                            out=table[r0:r0 + rem, 0:cfg.rowp],
                            in_=stg[:rem, ft, :])

            for n0 in p1_starts:
                p1_chunk(n0)

        # barrier: all table rows written before the main gathers read them
        tc.strict_bb_all_engine_barrier()

        # ---------------- phase 2: edge pass ----------------
        with (
            tc.tile_pool(name="p2g", bufs=3) as gpool,
            tc.tile_pool(name="p2s", bufs=3) as s1pool,
            tc.tile_pool(name="p2e", bufs=3) as epool,
            tc.tile_pool(name="p2o", bufs=3) as obpool,
            tc.tile_pool(name="p2ps", bufs=2, space="PSUM") as ps2pool,
            tc.tile_pool(name="p2bc", bufs=2, space="PSUM") as bcpool,
            tc.tile_pool(name="p2ad", bufs=2, space="PSUM") as adpool,
            tc.tile_pool(name="p2rl", bufs=3) as rlpool,
        ):
            # chunks per group (tile ranges are group-contiguous)
            grp_chunks = [[] for _ in meta["groups"]]
            bounds = []
            t_acc = 0
            for gi, blks in enumerate(meta["groups"]):
                span = sum(int(meta["tl"][b] + meta["th"][b]) for b in blks)
                bounds.append((t_acc, t_acc + span))
                t_acc += span
            for ch in meta["chunks"]:
                for gi, (g0, g1) in enumerate(bounds):
                    if g0 <= ch["t0"] < g1:
                        grp_chunks[gi].append(ch)
                        break

            def do_epilogue(b, psb):
                den = epool.tile([P, h], F32, tag="den")
                nc.vector.tensor_scalar_add(out=den[:],
                                            in0=psb[:, hc:hc + h],
                                            scalar1=1e-6)
                rec = epool.tile([P, h], F32, tag="rec")
                nc.vector.reciprocal(out=rec[:], in_=den[:])
                ob = obpool.tile([P, hc], F32, tag="ob")
                recb = AP(rec.tensor, rec[:].offset,
                          [rec[:].ap[0], [0, cfg.hid], [1, h]])
                nc.vector.tensor_tensor(
                    out=ob[:].rearrange("p (c hh) -> p c hh", hh=h),
                    in0=psb[:, 0:hc].rearrange("p (c hh) -> p c hh", hh=h),
                    in1=recb, op=OP.mult)
                if bias_nonzero:
                    nc.vector.tensor_add(out=ob[:], in0=ob[:],
                                         in1=bias_sb[:])
                t2 = obpool.tile([P, hc], F32, tag="t2")
                nc.vector.scalar_tensor_tensor(
                    out=t2[:], in0=ob[:], scalar=0.0, op0=OP.min,
                    in1=pw_sb[:], op1=OP.mult)
                obp = obpool.tile([P, hc], F32, tag="obp")
                obpv = obp[:]
                # write through a permuted view: col c*4+hh -> hh*64+c
                obp_perm = AP(obpv.tensor, obpv.offset,
                              [obpv.ap[0], [cfg.hid, h], [1, cfg.hid]])
                iview = [[1, h], [h, cfg.hid]]
                ob_i = AP(ob[:].tensor, ob[:].offset, [ob[:].ap[0]] + iview)
                t2_i = AP(t2[:].tensor, t2[:].offset, [t2[:].ap[0]] + iview)
                nc.vector.scalar_tensor_tensor(
                    out=obp_perm, in0=ob_i, scalar=0.0, op0=OP.max,
                    in1=t2_i, op1=OP.add)
                rows = min(P, cfg.ndst - b * P)
                nc.sync.dma_start(out=out[b * P:b * P + rows, :],
                                  in_=obp[:rows, :])

            for gi, blks in enumerate(meta["groups"]):
                ps_of = {}
                for b in blks:
                    psb = ps2pool.tile([P, hc + h], F32, tag="psb")
                    ps_of[b] = psb
                first_t = {b: meta["blk_span"][b][0] for b in blks}
                last_t = {b: meta["blk_span"][b][1] for b in blks}
                rl_of = {}
                if "adst" in parts:
                    for b in blks:
                        ncols = int(meta["tl"][b] + meta["th"][b]) * P
                        rl = rlpool.tile([1, meta["rtc"]], BF16, tag="rl")
                        nc.sync.dma_start(
                            out=rl[0:1, 0:ncols],
                            in_=pkv("relt", meta["rtc"], BF16,
                                    rows=(b, b + 1))[:, 0:ncols])
                        rl_of[b] = rl
                for chi, ch in enumerate(grp_chunks[gi]):
                    qq = ch["nt"]
                    t0 = ch["t0"]
                    g = gpool.tile([P, qq, cfg.gcol], BF16, tag="g")
                    if "gather" in parts:
                        if ch["half"] == "lo":
                            in_ap = table[0:cfg.win, :]
                        else:
                            in_ap = table[hi_base:cfg.rows, :]
                        nc.gpsimd.dma_gather(
                            out_ap=g[:],
                            in_ap=in_ap,
                            idxs_ap=midx_sb[:, ch["icol"]:ch["icol"]
                                            + ch["nidx"] // 16],
                            num_idxs=ch["nidx"],
                            num_idxs_reg=ch["nidx"],
                            elem_size=cfg.gcol,
                            elem_step=row,
                            queue_num=(gi + chi) % cfg.n_swq)

                    # one-hot S1[k, q, slot] = (rel[k, q] == slot)
                    s1 = s1pool.tile([P, qq, P], BF16, tag="s1")
                    if "s1" in parts:
                        rsl = rel_sb[:, 2 * t0:2 * (t0 + qq)]
                        rel_b = AP(rsl.tensor, rsl.offset,
                                   [rsl.ap[0], [2, qq], [0, P // 2], [1, 2]])
                        iap = iota_bf[:]
                        iota_b = AP(iap.tensor, iap.offset,
                                    [iap.ap[0], [0, qq], [2, P // 2], [1, 2]])
                        s1v = s1[:]
                        s1_b = AP(s1v.tensor, s1v.offset,
                                  [s1v.ap[0], [P, qq], [2, P // 2], [1, 2]])
                        nc.vector.tensor_tensor(
                            out=s1_b, in0=rel_b, in1=iota_b, op=OP.is_equal)

                    if "adst" in parts:
                        # per-edge a_dst without any DMA:
                        # 1) broadcast rel (slot id per edge) to all
                        #    partitions via PE outer product ones^T@rel_row
                        # 2) transposed one-hot S1T[slot,e]=(slot==rel[e])
                        # 3) a_dst_edge[e,:] = S1T^T @ adst_own[:,b,:]
                        b = _block_of(meta, ch)
                        bl0 = _blk_local(meta, ch)
                        s1t = s1pool.tile([P, qq, P], BF16, tag="s1t")
                        for j0 in range(0, qq, 4):
                            jj = min(4, qq - j0)
                            bc = bcpool.tile([P, 4 * P], F32, tag="bc")
                            c0 = (bl0 + j0) * P
                            nc.tensor.matmul(
                                bc[:, 0:jj * P],
                                lhsT=ones_row[0:1, :],
                                rhs=rl_of[b][0:1, c0:c0 + jj * P],
                                start=True, stop=True)
                            bcs = epool.tile([P, 4 * P], BF16, tag="bcs")
                            nc.scalar.copy(out=bcs[:, 0:jj * P],
                                           in_=bc[:, 0:jj * P])
                            ipb = AP(iotap_bf.tensor, iotap_bf[:].offset,
                                     [iotap_bf[:].ap[0], [0, jj], [0, P]])
                            s1tv = s1t[:, j0:j0 + jj, :]
                            nc.vector.tensor_tensor(
                                out=s1tv, in0=ipb,
                                in1=bcs[:, 0:jj * P].rearrange(
                                    "p (q e) -> p q e", q=jj),
                                op=OP.is_equal)
                        ps_ad = adpool.tile([P, qq, h], F32, tag="ps_ad")
                        for j in range(qq):
                            nc.tensor.matmul(
                                ps_ad[:, j, :],
                                lhsT=s1t[:, j, :],
                                rhs=adst_own[:, b, :],
                                start=True, stop=True)

                    if "pcomp" in parts:
                        # p = exp(leaky_relu(a_src + a_dst))
                        ef = epool.tile([P, qq, h], F32, tag="ef")
                        if "adst" in parts:
                            nc.vector.tensor_add(
                                out=ef[:],
                                in0=g[:, :, 4 + hc:4 + hc + h],
                                in1=ps_ad[:])
                        else:
                            nc.vector.tensor_copy(
                                out=ef[:], in_=g[:, :, 4 + hc:4 + hc + h])
                        # lrelu(e) = max(0.2*e, e) in one DVE op
                        nc.vector.scalar_tensor_tensor(
                            out=ef[:], in0=ef[:], scalar=NEG_SLOPE,
                            op0=OP.mult, in1=ef[:], op1=OP.max)
                        # exp on ACT, written straight into g as bf16
                        nc.scalar.activation(
                            out=g[:, :, 4 + hc:4 + hc + h], in_=ef[:],
                            func=AF.Exp)
                        # msg = h * p; h interleaved [c, hh] so the
                        # broadcast AP keeps a stride-1 last dim (2x DVE)
                        msg = g[:, :, 4:4 + hc].rearrange(
                            "p q (c hh) -> p q c hh", hh=h)
                        pslice = g[:, :, 4 + hc:4 + hc + h]
                        pb = AP(pslice.tensor, pslice.offset,
                                [pslice.ap[0], pslice.ap[1],
                                 [0, cfg.hid], [1, h]])
                        nc.vector.tensor_tensor(out=msg, in0=msg, in1=pb,
                                                op=OP.mult)

                    if "mm" in parts:
                        for j in range(qq):
                            tcol = t0 + j
                            b = next(bb for bb in blks
                                     if first_t[bb] <= tcol <= last_t[bb]
                                     and _tile_in_block(meta, bb, tcol))
                            nc.tensor.matmul(
                                ps_of[b][:, :],
                                lhsT=s1[:, j, :],
                                rhs=g[:, j, 4:4 + hc + h],
                                start=(tcol == first_t[b]),
                                stop=(tcol == last_t[b]))

                # epilogue: out = prelu(msg/denom + bias)
                if "epi" not in parts or "mm" not in parts:
                    continue
                for b in blks:
                    do_epilogue(b, ps_of[b][:])
        fpool_cm.__exit__(None, None, None)
    return nc


def _block_of(meta, ch):
    for (b, half), c0 in meta["col0"].items():
        if half != ch["half"]:
            continue
        cnt = meta["tl" if half == "lo" else "th"][b]
        if cnt and c0 <= ch["t0"] < c0 + int(cnt):
            assert ch["t0"] + ch["nt"] <= c0 + int(cnt), \
                "adst-mm path needs single-block chunks (cfg.group==1)"
            return b
    raise AssertionError(ch)


def _blk_local(meta, ch):
    b = _block_of(meta, ch)
    if ch["half"] == "lo":
        return ch["t0"] - meta["col0"][(b, "lo")]
    return int(meta["tl"][b]) + ch["t0"] - meta["col0"][(b, "hi")]


def _tile_in_block(meta, b, tcol):
    for half, cnt in (("lo", meta["tl"]), ("hi", meta["th"])):
        if cnt[b]:
            c0 = meta["col0"][(b, half)]
            if c0 <= tcol < c0 + int(cnt[b]):
                return True
    return False


# ---------------------------------------------------------------- runner

def _prepare(x, edge_index, lin_w, att_src, att_dst, bias, prelu_w, cfg,
             parts=None):
    per_core, meta = host_prep_edges(np.asarray(edge_index), cfg)
    shared = host_prep_weights(np.asarray(x), np.asarray(lin_w),
                               np.asarray(att_src), np.asarray(att_dst),
                               np.asarray(bias), np.asarray(prelu_w), cfg)
    bias_nonzero = bool(np.any(np.asarray(bias) != 0))
    nc = build(cfg, meta, bias_nonzero=bias_nonzero, parts=parts)
    in_maps = [dict(pk=pack_inputs(per_core[m], shared, m, cfg, meta))
               for m in range(cfg.n_cores)]
    return nc, in_maps


def _run_pjrt(nc, in_maps, n_cores, time_iters=0):
    """Mirror of bass2jax.run_bass_via_pjrt that keeps the compiled callable
    so warm re-executions can be timed (no NTFF profiling in this container).
    Returns (per-core result dicts, min warm wall ns or None)."""
    import time
    import jax
    from jax.sharding import Mesh, PartitionSpec
    from jax.experimental.shard_map import shard_map
    from concourse import bass2jax, mybir as mb

    bass2jax.install_neuronx_cc_hook()
    assert nc.dbg_addr is None
    partition_name = (nc.partition_id_tensor.name
                      if nc.partition_id_tensor else None)
    in_names, out_names, out_avals, zero_outs = [], [], [], []
    for alloc in nc.m.functions[0].allocations:
        if not isinstance(alloc, mb.MemoryLocationSet):
            continue
        name = alloc.memorylocations[0].name
        if alloc.kind == "ExternalInput":
            if name != partition_name:
                in_names.append(name)
        elif alloc.kind == "ExternalOutput":
            shape = tuple(alloc.tensor_shape)
            dtype = mb.dt.np(alloc.dtype)
            out_names.append(name)
            out_avals.append(jax.core.ShapedArray(shape, dtype))
            zero_outs.append(np.zeros(shape, dtype))
    n_params = len(in_names)
    in_names.extend(out_names)
    if partition_name is not None:
        in_names.append(partition_name)

    def _body(*args):
        operands = list(args)
        if partition_name is not None:
            operands.append(bass2jax.partition_id_tensor())
        outs = bass2jax._bass_exec_p.bind(
            *operands,
            out_avals=tuple(out_avals),
            in_names=tuple(in_names),
            out_names=tuple(out_names),
            lowering_input_output_aliases=(),
            sim_require_finite=True,
            sim_require_nnan=True,
            nc=nc,
        )
        return tuple(outs)

    devices = jax.devices()[:n_cores]
    mesh = Mesh(np.asarray(devices), ("core",))
    n_outs = len(out_avals)
    in_specs = (PartitionSpec("core"),) * (n_params + n_outs)
    out_specs = (PartitionSpec("core"),) * n_outs
    sharded = jax.jit(
        shard_map(_body, mesh=mesh, in_specs=in_specs, out_specs=out_specs,
                  check_rep=False),
        keep_unused=True,
    )
    per_core = [[np.asarray(m[name]) for name in in_names[:n_params]]
                for m in in_maps]
    concat_in = [
        np.concatenate([per_core[c][i] for c in range(n_cores)], axis=0)
        for i in range(n_params)
    ]
    concat_zeros = [
        np.zeros((n_cores * z.shape[0], *z.shape[1:]), z.dtype)
        for z in zero_outs
    ]
    sh = jax.sharding.NamedSharding(mesh, PartitionSpec("core"))
    dev_args = [jax.device_put(a, sh) for a in concat_in + concat_zeros]
    out_arrs = sharded(*dev_args)
    jax.block_until_ready(out_arrs)
    t_ns = None
    if time_iters > 0:
        # The axon RPC floor (~90 ms) swamps single-dispatch wall time, but
        # async dispatches pipeline on the device queue: time M back-to-back
        # executions blocking once, for two M values, and use the slope.
        def loop_wall(mreps):
            best = None
            for _ in range(time_iters):
                t0 = time.perf_counter_ns()
                o = None
                for _ in range(mreps):
                    o = sharded(*dev_args)
                jax.block_until_ready(o)
                dt = time.perf_counter_ns() - t0
                best = dt if best is None else min(best, dt)
            return best

        m1, m2 = 24, 72
        loop_wall(4)  # warm
        w1 = loop_wall(m1)
        w2 = loop_wall(m2)
        t_ns = max(0, (w2 - w1) // (m2 - m1))
    results = [
        {name: np.asarray(out_arrs[i]).reshape(n_cores, *out_avals[i].shape)[c]
         for i, name in enumerate(out_names)}
        for c in range(n_cores)
    ]
    return results, t_ns


def run(x, edge_index, lin_w, att_src, att_dst, bias, prelu_w,
        cfg=None, time_iters=0):
    cfg = cfg or CFG
    nc, in_maps = _prepare(x, edge_index, lin_w, att_src, att_dst, bias,
                           prelu_w, cfg)
    nc.finalize()
    results, t_ns = _run_pjrt(nc, in_maps, cfg.n_cores,
                              time_iters=time_iters)
    outs = [results[m]["out"] for m in range(cfg.n_cores)]
    full = np.concatenate(outs, axis=0).astype(np.float32)
    return full, t_ns


def kernel(**inputs):
    full, _ = run(inputs["x"], inputs["edge_index"], inputs["lin_w"],
                  inputs["att_src"], inputs["att_dst"], inputs["bias"],
                  inputs["prelu_w"])
    return full


# revision 40
# speedup vs baseline: 1.0015x; 1.0015x over previous
"""GAT layer (PyG GATConv H=4,C=64 + PReLU) on 8 Trainium2 NeuronCores.

Strategy (graph/data parallel, dst-sharded):
  - Host: add self loops, sort edges by dst, partition dst-nodes across 8
    cores (6250 each), group each core's edges into 128-dst "blocks", pair
    blocks into GROUPS, and tile each (group, window-half)'s edges into
    128-edge tiles.  Tile/chunk structure is uniform across cores so ONE
    SPMD program serves all 8 cores; per-core divergence rides in data.
  - Node rotation: core m's table stores node (m*6250 + l) mod N at row
    l+1, fed by a host-rotated copy of x.  Hence every core's OWN dst
    nodes are rows 1..6250 — the per-edge a_dst gather uses one small
    int16-indexed window identical on all cores.
  - Phase 1 (per core, replicated matmul): table row = [a_dst(4) | h(256)
    | a_src(4) | junk] (bf16, 768B stride).  h = x @ lin_w.T on PE;
    a_src/a_dst fold into the same matmul as extra columns (w_a =
    lin_w.T @ att).  Rows 0 and N+1 are sentinels with a_src=-30000 so
    padded edges get p = exp(lrelu(-big)) = 0.  Table writes are staged
    in SBUF per node-chunk and issued as ONE batched DMA per chunk on
    the scalar (ACT) HWDGE queue, overlapping the SP queue's x loads.
    PSUM->staging copies alternate between ACT and DVE.
  - Phase 2: per (block, half) chunk (up to tc_max tiles in one
    dma_gather; SWDGE ring enlarged via dynamic_dma_scratch_size),
    gather table rows by src, build one-hot S1[k,slot] by iota-compare,
    p = exp(leaky_relu(a_src+a_dst)) (lrelu on DVE via max(0.2e,e), exp
    on ACT written straight into the gathered row), messages h*p
    scatter-added into each block's 128 dst slots by matmul
    (lhsT=S1, rhs=[h*p | p]) accumulating [msg | denom] in PSUM.
    Epilogue per block: out = prelu(msg/denom + bias).
  - Per-edge a_dst needs NO DMA gather (the 256B-elem per-edge gather
    dominated the old kernel: sub-512B SDMA descriptors run at half
    rate).  Instead: a_dst of the core's own 6250 dst slots is stashed
    from phase-1 PSUM ([128, nblk, 4] in SBUF); per chunk, rel (slot id
    per edge) is broadcast to all partitions by a PE outer product
    ones^T @ rel_row (rel_row streamed to partition 0 per block), a
    transposed one-hot S1T[slot,e]=(slot==rel[e]) is built by DVE
    iota-compare, and a tiny PE matmul S1T^T @ adst_own[:,b,:] yields
    per-edge a_dst in PSUM, read directly by the DVE add.
  - All per-core inputs ride in ONE packed int16 tensor (midx | relT |
    rel | w | bias | prelu | sentinels | x pre-rearranged), bitcast
    per region on the DRAM side; 2 buffers per core total keeps the
    per-dispatch host cost down.
  - Softmax max-subtraction is skipped (logits are O(1); exp can't
    overflow) making the edge pass single-sweep: out = (Σ p·h)/(Σ p).
"""

import sys

sys.path.insert(0, "/opt/trn_rl_repo")

import numpy as np
import ml_dtypes

import concourse.bass as bass
import concourse.bacc as bacc
import concourse.tile as tile
from concourse import mybir
from concourse.bass import AP

F32 = mybir.dt.float32
BF16 = mybir.dt.bfloat16
I16 = mybir.dt.int16
AF = mybir.ActivationFunctionType
OP = mybir.AluOpType
BF16NP = ml_dtypes.bfloat16

P = 128
NEG_SLOPE = 0.2
SENT_NEG = -30000.0


class Cfg:
    def __init__(self, n_nodes=50000, in_ch=512, hid=64, heads=4, n_cores=8,
                 tc_max=8, node_chunk=2048, win=32768, group=1,
                 dma_scratch=49152):
        assert n_nodes % n_cores == 0
        assert in_ch % P == 0
        self.n_nodes = n_nodes
        self.in_ch = in_ch
        self.hid = hid
        self.heads = heads
        self.hc = hid * heads                      # 256
        self.rowp = self.hc + 2 * heads            # 264 payload cols
        self.row = 384                             # table row stride (768B)
        self.gcol = self.row                       # gather full 768B rows
        self.n_cores = n_cores
        self.ndst = n_nodes // n_cores             # 6250
        self.nblk = -(-self.ndst // P)             # 49
        self.kt = in_ch // P                       # 4
        self.tc_max = tc_max
        self.node_chunk = node_chunk
        self.rows = n_nodes + 2                    # + two sentinel rows
        self.win = min(win, self.rows)             # int16 gather window
        assert self.rows <= 2 * self.win, "lo+hi windows must cover table"
        self.group = group
        self.dma_scratch = dma_scratch
        self.ring = dma_scratch // 16              # SWDGE descriptor ring
        self.n_swq = 1                             # SWDGE queues (gathers)


CFG = Cfg()


def _wrap16(flat):
    """int16 index list -> dma_gather layout [128, n/16] (i -> [i%16, i//16],
    replicated to all 8 Q7 core groups)."""
    n = len(flat)
    assert n % 16 == 0
    a = np.asarray(flat, dtype=np.int16).reshape(n // 16, 16).T  # [16, n/16]
    return np.tile(a, (8, 1))                                    # [128, 128]


# ---------------------------------------------------------------- host prep

def host_prep_edges(edge_index, cfg):
    """Sort/partition/tile edges.  Returns (per_core data dicts, meta).

    Tile column space: for each group of `cfg.group` blocks, first all LO
    tiles (block-major), then all HI tiles.  Chunks (= one dma_gather each)
    split each (group, half) span at tc_max tiles.
    """
    n = cfg.n_nodes
    loop = np.arange(n, dtype=np.int64)
    src = np.concatenate([edge_index[0].astype(np.int64), loop])
    dst = np.concatenate([edge_index[1].astype(np.int64), loop])
    order = np.argsort(dst, kind="stable")
    src_s = src[order]
    dst_s = dst[order]

    lo_rows = cfg.win                 # lo window = rows [0, win)
    hi_base = cfg.rows - cfg.win      # hi window = rows [hi_base, rows)
    sent_hi_rel = cfg.rows - 1 - hi_base

    # per-(core, block) segments; rotated src rows; lo/hi split
    seg = {}
    tl_req = np.zeros((cfg.n_cores, cfg.nblk), dtype=np.int64)
    th_req = np.zeros((cfg.n_cores, cfg.nblk), dtype=np.int64)
    for m in range(cfg.n_cores):
        base = m * cfg.ndst
        for b in range(cfg.nblk):
            d0 = base + b * P
            d1 = min(base + (b + 1) * P, base + cfg.ndst)
            lo = np.searchsorted(dst_s, d0)
            hi = np.searchsorted(dst_s, d1)
            s_rot = (src_s[lo:hi] - base) % n + 1     # rotated table row
            d_loc = dst_s[lo:hi] - d0                 # slot in block
            a_idx = dst_s[lo:hi] - base + 1           # rotated a_dst row
            is_lo = s_rot < lo_rows
            seg[(m, b)] = (s_rot, d_loc, a_idx, is_lo)
            tl_req[m, b] = -(-int(is_lo.sum()) // P)
            th_req[m, b] = -(-int((~is_lo).sum()) // P)
    tl = tl_req.max(axis=0).astype(int)
    th = th_req.max(axis=0).astype(int)
    for b in range(cfg.nblk):
        if tl[b] + th[b] == 0:
            tl[b] = 1

    # ---- tile column layout: group-major, half-major, block-major
    ngrp = -(-cfg.nblk // cfg.group)
    groups = [list(range(g * cfg.group, min((g + 1) * cfg.group, cfg.nblk)))
              for g in range(ngrp)]
    col0 = {}            # (b, half) -> first tile column
    blk_span = {}        # b -> (t_first, t_last) inclusive tile cols
    chunks = []          # dicts: b-range via tiles list, t0, nt, icol, half
    icol = 0
    T = 0
    for blks in groups:
        for half in ("lo", "hi"):
            cnt = tl if half == "lo" else th
            span_t0 = T
            for b in blks:
                col0[(b, half)] = T
                T += int(cnt[b])
            span_nt = T - span_t0
            # split the span into gather chunks of <= tc_max tiles
            q0 = 0
            while q0 < span_nt:
                qq = min(cfg.tc_max, span_nt - q0)
                assert qq * P <= cfg.ring, (qq, cfg.ring)
                chunks.append(dict(t0=span_t0 + q0, nt=qq, nidx=qq * P,
                                   icol=icol, half=half))
                icol += qq * P // 16
                q0 += qq
    icol_main = icol
    for b in range(cfg.nblk):
        ts, te = [], []
        for half, cnt in (("lo", tl), ("hi", th)):
            if cnt[b]:
                ts.append(col0[(b, half)])
                te.append(col0[(b, half)] + int(cnt[b]) - 1)
        blk_span[b] = (min(ts), max(te))

    # relT layout: partition b holds block b's rel values (lo tiles then hi
    # tiles, matching the global tile-column order restricted to b)
    assert cfg.nblk <= P
    rtc = int((tl + th).max()) * P

    per_core = []
    for m in range(cfg.n_cores):
        rel_all = np.zeros((P, T), dtype=np.float32)
        relt = np.zeros((P, rtc), dtype=np.float32)
        midx_all = np.zeros((P, icol_main), dtype=np.int16)
        sidx_all = np.zeros((P, T * P // 16), dtype=np.int16)  # per-tile src
        for b in range(cfg.nblk):
            s_rot, d_loc, a_idx, is_lo = seg[(m, b)]
            boff = 0
            for half, cnt in (("lo", tl), ("hi", th)):
                nt_half = int(cnt[b])
                if nt_half == 0:
                    continue
                sel = is_lo if half == "lo" else ~is_lo
                ne = int(sel.sum())
                npad = nt_half * P
                if half == "lo":
                    bs = np.zeros(npad, dtype=np.int64)       # sentinel_lo
                else:
                    bs = np.full(npad, sent_hi_rel + hi_base, dtype=np.int64)
                br = np.zeros(npad, dtype=np.int64)
                bs[:ne] = s_rot[sel]
                br[:ne] = d_loc[sel]
                if half == "hi":
                    bs -= hi_base
                assert bs.min() >= 0 and bs.max() < cfg.win
                t0 = col0[(b, half)]
                rel_all[:, t0:t0 + nt_half] = br.reshape(nt_half, P).T
                relt[b, boff:boff + npad] = br
                boff += npad
                sidx_all[:, t0 * P // 16:(t0 + nt_half) * P // 16] = \
                    _wrap16(bs)
        for ch in chunks:
            c0, c1 = ch["t0"] * P // 16, (ch["t0"] + ch["nt"]) * P // 16
            midx_all[:, ch["icol"]:ch["icol"] + ch["nidx"] // 16] = \
                sidx_all[:, c0:c1]
        rel2 = np.repeat(rel_all.astype(BF16NP), 2, axis=1)   # [P, 2T]
        per_core.append(dict(
            midx=np.ascontiguousarray(midx_all),
            relt=np.ascontiguousarray(relt.astype(BF16NP)),
            rel=np.ascontiguousarray(rel2),
        ))
    meta = dict(chunks=chunks, tl=tl, th=th, col0=col0, blk_span=blk_span,
                groups=groups, T=T, icol_main=icol_main, rtc=rtc,
                hi_base=hi_base)
    meta["off"] = _pack_offsets(cfg, meta)
    return per_core, meta


def _pack_offsets(cfg, meta):
    """Column offsets (int16 units) of each region in the packed input."""
    off = {}
    c = 0
    for name, cols in (
        ("midx", meta["icol_main"]),
        ("relt", meta["rtc"]),
        ("rel", 2 * meta["T"]),
        ("w", cfg.kt * cfg.rowp),
        ("bias", 2 * cfg.hc),
        ("pw", 2 * cfg.hc),
        ("sent", cfg.rowp),
        ("xt", cfg.kt * cfg.n_nodes),
    ):
        off[name] = c
        c += cols
    off["end"] = c
    return off


def pack_inputs(pc, shared, m, cfg, meta):
    """One [128, PKC] int16 tensor holding every per-core input."""
    off = meta["off"]
    pk = np.zeros((P, off["end"]), dtype=np.int16)

    def put(name, arr):
        a = np.ascontiguousarray(arr).view(np.int16)
        pk[:a.shape[0], off[name]:off[name] + a.shape[1]] = a

    put("midx", pc["midx"])
    put("relt", pc["relt"])
    put("rel", pc["rel"])
    # rhs_w pre-transposed to the matmul layout: w[p, k*rowp + r]
    rw = shared["rhs_w"]                      # [in_ch, rowp] bf16
    wt = rw.reshape(cfg.kt, P, cfg.rowp).transpose(1, 0, 2).reshape(
        P, cfg.kt * cfg.rowp)
    put("w", wt)
    put("bias", shared["bias_rep"])
    put("pw", shared["pw_rep"])
    put("sent", shared["sent"])
    # xT pre-rearranged to the xg layout: x[p, k*n + q] = xT[k*128+p, q]
    xr = shared["xTs"][m].reshape(cfg.kt, P, cfg.n_nodes).transpose(
        1, 0, 2).reshape(P, cfg.kt * cfg.n_nodes)
    put("xt", xr)
    return pk


def host_prep_weights(x, lin_w, att_src, att_dst, bias, prelu_w, cfg):
    n, ic, h, c = cfg.n_nodes, cfg.in_ch, cfg.heads, cfg.hid
    w3 = lin_w.astype(np.float64).reshape(h, c, ic)
    wa_src = (w3 * att_src.astype(np.float64).reshape(h, c, 1)).sum(1).T
    wa_dst = (w3 * att_dst.astype(np.float64).reshape(h, c, 1)).sum(1).T
    lwT = lin_w.astype(np.float64).T                           # [ic, 256]
    lwTi = lwT.reshape(ic, h, c).transpose(0, 2, 1).reshape(ic, h * c)
    rhs = np.concatenate([wa_dst, lwTi, wa_src], axis=1)       # [ic, 264]
    rhs_w = np.ascontiguousarray(rhs.astype(BF16NP))
    def inter(v):
        return v.reshape(h, c).T.reshape(h * c)
    bias_rep = np.ascontiguousarray(np.broadcast_to(
        inter(bias.astype(np.float32)), (P, cfg.hc)))
    pw_rep = np.ascontiguousarray(np.broadcast_to(
        inter(prelu_w.astype(np.float32)), (P, cfg.hc)))
    sent = np.zeros((2, cfg.rowp), dtype=BF16NP)
    sent[:, cfg.rowp - cfg.heads:] = SENT_NEG      # a_src cols
    xbf = x.astype(BF16NP)
    xTs = []
    for m in range(cfg.n_cores):
        r = np.roll(xbf, -m * cfg.ndst, axis=0)
        xTs.append(np.ascontiguousarray(r.T))
    return dict(rhs_w=rhs_w, bias_rep=bias_rep, pw_rep=pw_rep, sent=sent,
                xTs=xTs)


# ---------------------------------------------------------------- builder

def build(cfg, meta, bias_nonzero=True, parts=None):
    parts = parts if parts is not None else {
        "p1", "gather", "adst", "s1", "pcomp", "mm", "epi"}
    if "p1" in parts:
        parts |= {"p1x", "p1mm", "p1w"}
    n, row, hc, h = cfg.n_nodes, cfg.row, cfg.hc, cfg.heads
    nc = bacc.Bacc(dynamic_dma_scratch_size=cfg.dma_scratch,
                   num_swdge_queues=cfg.n_swq)

    off = meta["off"]
    pk = nc.declare_dram_parameter("pk", [P, off["end"]], I16,
                                   isOutput=False)
    out = nc.declare_dram_parameter("out", [cfg.ndst, hc], F32, isOutput=True)

    def pkv(name, cols, dt=None, rows=(0, P)):
        v = pk[rows[0]:rows[1], off[name]:off[name] + cols]
        return v.bitcast(dt) if dt is not None else v

    table = nc.dram_tensor("table", [cfg.rows, row], BF16)
    T = meta["T"]
    hi_base = meta["hi_base"]

    with tile.TileContext(nc) as tc:
        fpool_cm = tc.tile_pool(name="p2f", bufs=1)
        fpool = fpool_cm.__enter__()

        midx_sb = fpool.tile([P, meta["icol_main"]], I16)
        nc.sync.dma_start(out=midx_sb[:], in_=pkv("midx", meta["icol_main"]))
        rel_sb = fpool.tile([P, 2 * T], BF16)
        nc.sync.dma_start(out=rel_sb[:], in_=pkv("rel", 2 * T, BF16))
        bias_sb = fpool.tile([P, hc], F32)
        nc.sync.dma_start(out=bias_sb[:], in_=pkv("bias", 2 * hc, F32))
        pw_sb = fpool.tile([P, hc], F32)
        nc.sync.dma_start(out=pw_sb[:], in_=pkv("pw", 2 * hc, F32))

        iota_i = fpool.tile([P, P], mybir.dt.int32)
        nc.gpsimd.iota(iota_i[:], pattern=[[1, P]], base=0,
                       channel_multiplier=0)
        iota_f = fpool.tile([P, P], F32)
        nc.vector.tensor_copy(out=iota_f[:], in_=iota_i[:])
        iota_bf = fpool.tile([P, P], BF16)
        nc.vector.tensor_copy(out=iota_bf[:], in_=iota_f[:])
        # per-partition index column (for the transposed one-hot)
        iotap_i = fpool.tile([P, 1], mybir.dt.int32)
        nc.gpsimd.iota(iotap_i[:], pattern=[[0, 1]], base=0,
                       channel_multiplier=1)
        iotap_f = fpool.tile([P, 1], F32)
        nc.vector.tensor_copy(out=iotap_f[:], in_=iotap_i[:])
        # all-ones row for the PE partition-broadcast (ones^T @ rel_row)
        ones_row = fpool.tile([1, P], BF16)
        nc.vector.memset(ones_row[:], 1.0)
        # per-block a_dst of this core's own dst slots, filled in phase 1
        adst_own = fpool.tile([P, cfg.nblk, h], BF16)

        # ---------------- phase 1: build table ----------------
        with (
            tc.tile_pool(name="p1w", bufs=1) as wpool,
            tc.tile_pool(name="p1x", bufs=2) as xpool,
            tc.tile_pool(name="p1o", bufs=2) as opool,
            tc.tile_pool(name="p1ps", bufs=4, space="PSUM") as pspool,
        ):
            w_sb = wpool.tile([P, cfg.kt, cfg.rowp], BF16)
            nc.sync.dma_start(
                out=w_sb[:],
                in_=pkv("w", cfg.kt * cfg.rowp, BF16).rearrange(
                    "p (k r) -> p k r", k=cfg.kt))
            nc.sync.dma_start(out=table[0:1, 0:cfg.rowp],
                              in_=pkv("sent", cfg.rowp, BF16, rows=(0, 1)))
            nc.sync.dma_start(out=table[cfg.rows - 1:cfg.rows, 0:cfg.rowp],
                              in_=pkv("sent", cfg.rowp, BF16, rows=(1, 2)))

            nch = cfg.node_chunk
            p1_any = parts & {"p1x", "p1mm", "p1w"}
            p1_starts = list(range(0, n if p1_any else 0, nch))

            def p1_chunk(n0):
                nn = min(nch, n - n0)
                nt = -(-nn // P)
                xg = xpool.tile([P, cfg.kt, nch], BF16, tag="xg")
                if "p1x" in parts:
                    xtv = pkv("xt", cfg.kt * n, BF16).rearrange(
                        "p (k q) -> p k q", k=cfg.kt)
                    nc.sync.dma_start(
                        out=xg[:, :, :nn], in_=xtv[:, :, n0:n0 + nn])
                stg = opool.tile([P, nch // P, cfg.rowp], BF16, tag="stg")
                for ti in range(nt):
                    t0 = ti * P
                    mm = min(P, nn - t0)
                    if "p1mm" in parts:
                        ps = pspool.tile([P, cfg.rowp], F32, tag="ps")
                        for k in range(cfg.kt):
                            nc.tensor.matmul(
                                ps[:mm, :],
                                lhsT=xg[:, k, t0:t0 + mm],
                                rhs=w_sb[:, k, :],
                                start=(k == 0), stop=(k == cfg.kt - 1))
                        if ti % 3 == 2:
                            nc.vector.tensor_copy(out=stg[:mm, ti, :],
                                                  in_=ps[:mm, :])
                        else:
                            nc.scalar.copy(out=stg[:mm, ti, :],
                                           in_=ps[:mm, :])
                        # own-block a_dst (table rows 1+128b..) align with
                        # the psum tile grid: stash cols 0:4 for phase 2
                        gt = (n0 + t0) // P
                        if gt < cfg.nblk:
                            nc.vector.tensor_copy(
                                out=adst_own[:, gt, :], in_=ps[:, 0:h])
                if "p1w" in parts:
                    # one batched write: DRAM rows [1+n0, 1+n0+nn) cols
                    # 0:rowp at 768B row stride, via the ACT HWDGE queue.
                    # A partial last tile is written separately so garbage
                    # staging rows never land past the table.
                    ft, rem = divmod(nn, P)
                    if ft:
                        tv = table[1 + n0:1 + n0 + ft * P, 0:cfg.rowp]
                        dst = AP(tv.tensor, tv.offset,
                                 [[row, P], [P * row, ft], [1, cfg.rowp]])
                        nc.scalar.dma_start(out=dst, in_=stg[:, :ft, :])
                    if rem:
                        r0 = 1 + n0 + ft * P
                        nc.scalar.dma_start(
                            out=table[r0:r0 + rem, 0:cfg.rowp],
                            in_=stg[:rem, ft, :])

            for n0 in p1_starts:
                p1_chunk(n0)

        # barrier: all table rows written before the main gathers read them
        tc.strict_bb_all_engine_barrier()

        # ---------------- phase 2: edge pass ----------------
        with (
            tc.tile_pool(name="p2g", bufs=3) as gpool,
            tc.tile_pool(name="p2s", bufs=3) as s1pool,
            tc.tile_pool(name="p2e", bufs=3) as epool,
            tc.tile_pool(name="p2o", bufs=3) as obpool,
            tc.tile_pool(name="p2ps", bufs=2, space="PSUM") as ps2pool,
            tc.tile_pool(name="p2bc", bufs=2, space="PSUM") as bcpool,
            tc.tile_pool(name="p2ad", bufs=2, space="PSUM") as adpool,
            tc.tile_pool(name="p2rl", bufs=3) as rlpool,
        ):
            # chunks per group (tile ranges are group-contiguous)
            grp_chunks = [[] for _ in meta["groups"]]
            bounds = []
            t_acc = 0
            for gi, blks in enumerate(meta["groups"]):
                span = sum(int(meta["tl"][b] + meta["th"][b]) for b in blks)
                bounds.append((t_acc, t_acc + span))
                t_acc += span
            for ch in meta["chunks"]:
                for gi, (g0, g1) in enumerate(bounds):
                    if g0 <= ch["t0"] < g1:
                        grp_chunks[gi].append(ch)
                        break

            def do_epilogue(b, psb):
                den = epool.tile([P, h], F32, tag="den")
                nc.vector.tensor_scalar_add(out=den[:],
                                            in0=psb[:, hc:hc + h],
                                            scalar1=1e-6)
                rec = epool.tile([P, h], F32, tag="rec")
                nc.vector.reciprocal(out=rec[:], in_=den[:])
                ob = obpool.tile([P, hc], F32, tag="ob")
                recb = AP(rec.tensor, rec[:].offset,
                          [rec[:].ap[0], [0, cfg.hid], [1, h]])
                nc.vector.tensor_tensor(
                    out=ob[:].rearrange("p (c hh) -> p c hh", hh=h),
                    in0=psb[:, 0:hc].rearrange("p (c hh) -> p c hh", hh=h),
                    in1=recb, op=OP.mult)
                if bias_nonzero:
                    nc.vector.tensor_add(out=ob[:], in0=ob[:],
                                         in1=bias_sb[:])
                t2 = obpool.tile([P, hc], F32, tag="t2")
                nc.vector.scalar_tensor_tensor(
                    out=t2[:], in0=ob[:], scalar=0.0, op0=OP.min,
                    in1=pw_sb[:], op1=OP.mult)
                obp = obpool.tile([P, hc], F32, tag="obp")
                obpv = obp[:]
                # write through a permuted view: col c*4+hh -> hh*64+c
                obp_perm = AP(obpv.tensor, obpv.offset,
                              [obpv.ap[0], [cfg.hid, h], [1, cfg.hid]])
                iview = [[1, h], [h, cfg.hid]]
                ob_i = AP(ob[:].tensor, ob[:].offset, [ob[:].ap[0]] + iview)
                t2_i = AP(t2[:].tensor, t2[:].offset, [t2[:].ap[0]] + iview)
                nc.vector.scalar_tensor_tensor(
                    out=obp_perm, in0=ob_i, scalar=0.0, op0=OP.max,
                    in1=t2_i, op1=OP.add)
                rows = min(P, cfg.ndst - b * P)
                nc.sync.dma_start(out=out[b * P:b * P + rows, :],
                                  in_=obp[:rows, :])

            for gi, blks in enumerate(meta["groups"]):
                ps_of = {}
                for b in blks:
                    psb = ps2pool.tile([P, hc + h], F32, tag="psb")
                    ps_of[b] = psb
                first_t = {b: meta["blk_span"][b][0] for b in blks}
                last_t = {b: meta["blk_span"][b][1] for b in blks}
                rl_of = {}
                if "adst" in parts:
                    for b in blks:
                        ncols = int(meta["tl"][b] + meta["th"][b]) * P
                        rl = rlpool.tile([1, meta["rtc"]], BF16, tag="rl")
                        nc.sync.dma_start(
                            out=rl[0:1, 0:ncols],
                            in_=pkv("relt", meta["rtc"], BF16,
                                    rows=(b, b + 1))[:, 0:ncols])
                        rl_of[b] = rl
                for chi, ch in enumerate(grp_chunks[gi]):
                    qq = ch["nt"]
                    t0 = ch["t0"]
                    g = gpool.tile([P, qq, cfg.gcol], BF16, tag="g")
                    if "gather" in parts:
                        if ch["half"] == "lo":
                            in_ap = table[0:cfg.win, :]
                        else:
                            in_ap = table[hi_base:cfg.rows, :]
                        nc.gpsimd.dma_gather(
                            out_ap=g[:],
                            in_ap=in_ap,
                            idxs_ap=midx_sb[:, ch["icol"]:ch["icol"]
                                            + ch["nidx"] // 16],
                            num_idxs=ch["nidx"],
                            num_idxs_reg=ch["nidx"],
                            elem_size=cfg.gcol,
                            elem_step=row,
                            queue_num=(gi + chi) % cfg.n_swq)

                    # one-hot S1[k, q, slot] = (rel[k, q] == slot)
                    s1 = s1pool.tile([P, qq, P], BF16, tag="s1")
                    if "s1" in parts:
                        rsl = rel_sb[:, 2 * t0:2 * (t0 + qq)]
                        rel_b = AP(rsl.tensor, rsl.offset,
                                   [rsl.ap[0], [2, qq], [0, P // 2], [1, 2]])
                        iap = iota_bf[:]
                        iota_b = AP(iap.tensor, iap.offset,
                                    [iap.ap[0], [0, qq], [2, P // 2], [1, 2]])
                        s1v = s1[:]
                        s1_b = AP(s1v.tensor, s1v.offset,
                                  [s1v.ap[0], [P, qq], [2, P // 2], [1, 2]])
                        nc.vector.tensor_tensor(
                            out=s1_b, in0=rel_b, in1=iota_b, op=OP.is_equal)

                    if "adst" in parts:
                        # per-edge a_dst without any DMA:
                        # 1) broadcast rel (slot id per edge) to all
                        #    partitions via PE outer product ones^T@rel_row
                        # 2) transposed one-hot S1T[slot,e]=(slot==rel[e])
                        # 3) a_dst_edge[e,:] = S1T^T @ adst_own[:,b,:]
                        b = _block_of(meta, ch)
                        bl0 = _blk_local(meta, ch)
                        s1t = s1pool.tile([P, qq, P], BF16, tag="s1t")
                        for j0 in range(0, qq, 4):
                            jj = min(4, qq - j0)
                            bc = bcpool.tile([P, 4 * P], F32, tag="bc")
                            c0 = (bl0 + j0) * P
                            nc.tensor.matmul(
                                bc[:, 0:jj * P],
                                lhsT=ones_row[0:1, :],
                                rhs=rl_of[b][0:1, c0:c0 + jj * P],
                                start=True, stop=True)
                            ipb = AP(iotap_f.tensor, iotap_f[:].offset,
                                     [iotap_f[:].ap[0], [0, jj], [0, P]])
                            s1tv = s1t[:, j0:j0 + jj, :]
                            nc.vector.tensor_tensor(
                                out=s1tv, in0=ipb,
                                in1=bc[:, 0:jj * P].rearrange(
                                    "p (q e) -> p q e", q=jj),
                                op=OP.is_equal)
                        ps_ad = adpool.tile([P, qq, h], F32, tag="ps_ad")
                        for j in range(qq):
                            nc.tensor.matmul(
                                ps_ad[:, j, :],
                                lhsT=s1t[:, j, :],
                                rhs=adst_own[:, b, :],
                                start=True, stop=True)

                    if "pcomp" in parts:
                        # p = exp(leaky_relu(a_src + a_dst))
                        ef = epool.tile([P, qq, h], F32, tag="ef")
                        if "adst" in parts:
                            nc.vector.tensor_add(
                                out=ef[:],
                                in0=g[:, :, 4 + hc:4 + hc + h],
                                in1=ps_ad[:])
                        else:
                            nc.vector.tensor_copy(
                                out=ef[:], in_=g[:, :, 4 + hc:4 + hc + h])
                        # lrelu(e) = max(0.2*e, e) in one DVE op
                        nc.vector.scalar_tensor_tensor(
                            out=ef[:], in0=ef[:], scalar=NEG_SLOPE,
                            op0=OP.mult, in1=ef[:], op1=OP.max)
                        # exp on ACT, written straight into g as bf16
                        nc.scalar.activation(
                            out=g[:, :, 4 + hc:4 + hc + h], in_=ef[:],
                            func=AF.Exp)
                        # msg = h * p; h interleaved [c, hh] so the
                        # broadcast AP keeps a stride-1 last dim (2x DVE)
                        msg = g[:, :, 4:4 + hc].rearrange(
                            "p q (c hh) -> p q c hh", hh=h)
                        pslice = g[:, :, 4 + hc:4 + hc + h]
                        pb = AP(pslice.tensor, pslice.offset,
                                [pslice.ap[0], pslice.ap[1],
                                 [0, cfg.hid], [1, h]])
                        nc.vector.tensor_tensor(out=msg, in0=msg, in1=pb,
                                                op=OP.mult)

                    if "mm" in parts:
                        for j in range(qq):
                            tcol = t0 + j
                            b = next(bb for bb in blks
                                     if first_t[bb] <= tcol <= last_t[bb]
                                     and _tile_in_block(meta, bb, tcol))
                            nc.tensor.matmul(
                                ps_of[b][:, :],
                                lhsT=s1[:, j, :],
                                rhs=g[:, j, 4:4 + hc + h],
                                start=(tcol == first_t[b]),
                                stop=(tcol == last_t[b]))

                # epilogue: out = prelu(msg/denom + bias)
                if "epi" not in parts or "mm" not in parts:
                    continue
                for b in blks:
                    do_epilogue(b, ps_of[b][:])
        fpool_cm.__exit__(None, None, None)
    return nc


def _block_of(meta, ch):
    for (b, half), c0 in meta["col0"].items():
        if half != ch["half"]:
            continue
        cnt = meta["tl" if half == "lo" else "th"][b]
        if cnt and c0 <= ch["t0"] < c0 + int(cnt):
            assert ch["t0"] + ch["nt"] <= c0 + int(cnt), \
                "adst-mm path needs single-block chunks (cfg.group==1)"
            return b
    raise AssertionError(ch)


def _blk_local(meta, ch):
    b = _block_of(meta, ch)
    if ch["half"] == "lo":
        return ch["t0"] - meta["col0"][(b, "lo")]
    return int(meta["tl"][b]) + ch["t0"] - meta["col0"][(b, "hi")]


def _tile_in_block(meta, b, tcol):
    for half, cnt in (("lo", meta["tl"]), ("hi", meta["th"])):
        if cnt[b]:
            c0 = meta["col0"][(b, half)]
            if c0 <= tcol < c0 + int(cnt[b]):
                return True
    return False


# ---------------------------------------------------------------- runner

def _prepare(x, edge_index, lin_w, att_src, att_dst, bias, prelu_w, cfg,
             parts=None):
    per_core, meta = host_prep_edges(np.asarray(edge_index), cfg)
    shared = host_prep_weights(np.asarray(x), np.asarray(lin_w),
                               np.asarray(att_src), np.asarray(att_dst),
                               np.asarray(bias), np.asarray(prelu_w), cfg)
    bias_nonzero = bool(np.any(np.asarray(bias) != 0))
    nc = build(cfg, meta, bias_nonzero=bias_nonzero, parts=parts)
    in_maps = [dict(pk=pack_inputs(per_core[m], shared, m, cfg, meta))
               for m in range(cfg.n_cores)]
    return nc, in_maps


def _run_pjrt(nc, in_maps, n_cores, time_iters=0):
    """Mirror of bass2jax.run_bass_via_pjrt that keeps the compiled callable
    so warm re-executions can be timed (no NTFF profiling in this container).
    Returns (per-core result dicts, min warm wall ns or None)."""
    import time
    import jax
    from jax.sharding import Mesh, PartitionSpec
    from jax.experimental.shard_map import shard_map
    from concourse import bass2jax, mybir as mb

    bass2jax.install_neuronx_cc_hook()
    assert nc.dbg_addr is None
    partition_name = (nc.partition_id_tensor.name
                      if nc.partition_id_tensor else None)
    in_names, out_names, out_avals, zero_outs = [], [], [], []
    for alloc in nc.m.functions[0].allocations:
        if not isinstance(alloc, mb.MemoryLocationSet):
            continue
        name = alloc.memorylocations[0].name
        if alloc.kind == "ExternalInput":
            if name != partition_name:
                in_names.append(name)
        elif alloc.kind == "ExternalOutput":
            shape = tuple(alloc.tensor_shape)
            dtype = mb.dt.np(alloc.dtype)
            out_names.append(name)
            out_avals.append(jax.core.ShapedArray(shape, dtype))
            zero_outs.append(np.zeros(shape, dtype))
    n_params = len(in_names)
    in_names.extend(out_names)
    if partition_name is not None:
        in_names.append(partition_name)

    def _body(*args):
        operands = list(args)
        if partition_name is not None:
            operands.append(bass2jax.partition_id_tensor())
        outs = bass2jax._bass_exec_p.bind(
            *operands,
            out_avals=tuple(out_avals),
            in_names=tuple(in_names),
            out_names=tuple(out_names),
            lowering_input_output_aliases=(),
            sim_require_finite=True,
            sim_require_nnan=True,
            nc=nc,
        )
        return tuple(outs)

    devices = jax.devices()[:n_cores]
    mesh = Mesh(np.asarray(devices), ("core",))
    n_outs = len(out_avals)
    in_specs = (PartitionSpec("core"),) * (n_params + n_outs)
    out_specs = (PartitionSpec("core"),) * n_outs
    sharded = jax.jit(
        shard_map(_body, mesh=mesh, in_specs=in_specs, out_specs=out_specs,
                  check_rep=False),
        keep_unused=True,
    )
    per_core = [[np.asarray(m[name]) for name in in_names[:n_params]]
                for m in in_maps]
    concat_in = [
        np.concatenate([per_core[c][i] for c in range(n_cores)], axis=0)
        for i in range(n_params)
    ]
    concat_zeros = [
        np.zeros((n_cores * z.shape[0], *z.shape[1:]), z.dtype)
        for z in zero_outs
    ]
    sh = jax.sharding.NamedSharding(mesh, PartitionSpec("core"))
    dev_args = [jax.device_put(a, sh) for a in concat_in + concat_zeros]
    out_arrs = sharded(*dev_args)
    jax.block_until_ready(out_arrs)
    t_ns = None
    if time_iters > 0:
        # The axon RPC floor (~90 ms) swamps single-dispatch wall time, but
        # async dispatches pipeline on the device queue: time M back-to-back
        # executions blocking once, for two M values, and use the slope.
        def loop_wall(mreps):
            best = None
            for _ in range(time_iters):
                t0 = time.perf_counter_ns()
                o = None
                for _ in range(mreps):
                    o = sharded(*dev_args)
                jax.block_until_ready(o)
                dt = time.perf_counter_ns() - t0
                best = dt if best is None else min(best, dt)
            return best

        m1, m2 = 24, 72
        loop_wall(4)  # warm
        w1 = loop_wall(m1)
        w2 = loop_wall(m2)
        t_ns = max(0, (w2 - w1) // (m2 - m1))
    results = [
        {name: np.asarray(out_arrs[i]).reshape(n_cores, *out_avals[i].shape)[c]
         for i, name in enumerate(out_names)}
        for c in range(n_cores)
    ]
    return results, t_ns


def run(x, edge_index, lin_w, att_src, att_dst, bias, prelu_w,
        cfg=None, time_iters=0):
    cfg = cfg or CFG
    nc, in_maps = _prepare(x, edge_index, lin_w, att_src, att_dst, bias,
                           prelu_w, cfg)
    nc.finalize()
    results, t_ns = _run_pjrt(nc, in_maps, cfg.n_cores,
                              time_iters=time_iters)
    outs = [results[m]["out"] for m in range(cfg.n_cores)]
    full = np.concatenate(outs, axis=0).astype(np.float32)
    return full, t_ns


def kernel(**inputs):
    full, _ = run(inputs["x"], inputs["edge_index"], inputs["lin_w"],
                  inputs["att_src"], inputs["att_dst"], inputs["bias"],
                  inputs["prelu_w"])
    return full


# revision 41
# speedup vs baseline: 1.0829x; 1.0813x over previous
"""GAT layer (PyG GATConv H=4,C=64 + PReLU) on 8 Trainium2 NeuronCores.

Strategy (graph/data parallel, dst-sharded):
  - Host: add self loops, sort edges by dst, partition dst-nodes across 8
    cores (6250 each), group each core's edges into 128-dst "blocks", pair
    blocks into GROUPS, and tile each (group, window-half)'s edges into
    128-edge tiles.  Tile/chunk structure is uniform across cores so ONE
    SPMD program serves all 8 cores; per-core divergence rides in data.
  - Node rotation: core m's table stores node (m*6250 + l) mod N at row
    l+1, fed by a host-rotated copy of x.  Hence every core's OWN dst
    nodes are rows 1..6250 — the per-edge a_dst gather uses one small
    int16-indexed window identical on all cores.
  - Phase 1 (per core, replicated matmul): table row = [a_dst(4) | h(256)
    | a_src(4) | junk] (bf16, 768B stride).  h = x @ lin_w.T on PE;
    a_src/a_dst fold into the same matmul as extra columns (w_a =
    lin_w.T @ att).  Rows 0 and N+1 are sentinels with a_src=-30000 so
    padded edges get p = exp(lrelu(-big)) = 0.  Table writes are staged
    in SBUF per node-chunk and issued as ONE batched DMA per chunk on
    the scalar (ACT) HWDGE queue, overlapping the SP queue's x loads.
    PSUM->staging copies alternate between ACT and DVE.
  - Phase 2: per (block, half) chunk (up to tc_max tiles in one
    dma_gather; SWDGE ring enlarged via dynamic_dma_scratch_size),
    gather table rows by src, build one-hot S1[k,slot] by iota-compare,
    p = exp(leaky_relu(a_src+a_dst)) (lrelu on DVE via max(0.2e,e), exp
    on ACT written straight into the gathered row), messages h*p
    scatter-added into each block's 128 dst slots by matmul
    (lhsT=S1, rhs=[h*p | p]) accumulating [msg | denom] in PSUM.
    Epilogue per block: out = prelu(msg/denom + bias).
  - Per-edge a_dst needs NO DMA gather (the 256B-elem per-edge gather
    dominated the old kernel: sub-512B SDMA descriptors run at half
    rate).  Instead: a_dst of the core's own 6250 dst slots is stashed
    from phase-1 PSUM ([128, nblk, 4] in SBUF); per chunk, rel (slot id
    per edge) is broadcast to all partitions by a PE outer product
    ones^T @ rel_row (rel_row streamed to partition 0 per block), a
    transposed one-hot S1T[slot,e]=(slot==rel[e]) is built by DVE
    iota-compare, and a tiny PE matmul S1T^T @ adst_own[:,b,:] yields
    per-edge a_dst in PSUM, read directly by the DVE add.
  - All per-core inputs ride in ONE packed int16 tensor (midx | relT |
    rel | w | bias | prelu | sentinels | x pre-rearranged), bitcast
    per region on the DRAM side; 2 buffers per core total keeps the
    per-dispatch host cost down.
  - Softmax max-subtraction is skipped (logits are O(1); exp can't
    overflow) making the edge pass single-sweep: out = (Σ p·h)/(Σ p).
"""

import sys

sys.path.insert(0, "/opt/trn_rl_repo")

import numpy as np
import ml_dtypes

import concourse.bass as bass
import concourse.bacc as bacc
import concourse.tile as tile
from concourse import mybir
from concourse.bass import AP

F32 = mybir.dt.float32
BF16 = mybir.dt.bfloat16
I16 = mybir.dt.int16
AF = mybir.ActivationFunctionType
OP = mybir.AluOpType
BF16NP = ml_dtypes.bfloat16

P = 128
NEG_SLOPE = 0.2
SENT_NEG = -30000.0


class Cfg:
    def __init__(self, n_nodes=50000, in_ch=512, hid=64, heads=4, n_cores=8,
                 tc_max=8, node_chunk=4096, win=32768, group=1,
                 dma_scratch=32768):
        assert n_nodes % n_cores == 0
        assert in_ch % P == 0
        self.n_nodes = n_nodes
        self.in_ch = in_ch
        self.hid = hid
        self.heads = heads
        self.hc = hid * heads                      # 256
        self.rowp = self.hc + 2 * heads            # 264 payload cols
        self.row = 384                             # table row stride (768B)
        self.gcol = self.row                       # gather full 768B rows
        self.n_cores = n_cores
        self.ndst = n_nodes // n_cores             # 6250
        self.nblk = -(-self.ndst // P)             # 49
        self.kt = in_ch // P                       # 4
        self.tc_max = tc_max
        self.node_chunk = node_chunk
        self.rows = n_nodes + 2                    # + two sentinel rows
        self.win = min(win, self.rows)             # int16 gather window
        assert self.rows <= 2 * self.win, "lo+hi windows must cover table"
        self.group = group
        self.dma_scratch = dma_scratch
        self.ring = dma_scratch // 16              # SWDGE descriptor ring
        self.n_swq = 1                             # SWDGE queues (gathers)


CFG = Cfg()


def _wrap16(flat):
    """int16 index list -> dma_gather layout [128, n/16] (i -> [i%16, i//16],
    replicated to all 8 Q7 core groups)."""
    n = len(flat)
    assert n % 16 == 0
    a = np.asarray(flat, dtype=np.int16).reshape(n // 16, 16).T  # [16, n/16]
    return np.tile(a, (8, 1))                                    # [128, 128]


# ---------------------------------------------------------------- host prep

def host_prep_edges(edge_index, cfg):
    """Sort/partition/tile edges.  Returns (per_core data dicts, meta).

    Tile column space: for each group of `cfg.group` blocks, first all LO
    tiles (block-major), then all HI tiles.  Chunks (= one dma_gather each)
    split each (group, half) span at tc_max tiles.
    """
    n = cfg.n_nodes
    loop = np.arange(n, dtype=np.int64)
    src = np.concatenate([edge_index[0].astype(np.int64), loop])
    dst = np.concatenate([edge_index[1].astype(np.int64), loop])
    order = np.argsort(dst, kind="stable")
    src_s = src[order]
    dst_s = dst[order]

    lo_rows = cfg.win                 # lo window = rows [0, win)
    hi_base = cfg.rows - cfg.win      # hi window = rows [hi_base, rows)
    sent_hi_rel = cfg.rows - 1 - hi_base

    # per-(core, block) segments; rotated src rows; lo/hi split
    seg = {}
    tl_req = np.zeros((cfg.n_cores, cfg.nblk), dtype=np.int64)
    th_req = np.zeros((cfg.n_cores, cfg.nblk), dtype=np.int64)
    for m in range(cfg.n_cores):
        base = m * cfg.ndst
        for b in range(cfg.nblk):
            d0 = base + b * P
            d1 = min(base + (b + 1) * P, base + cfg.ndst)
            lo = np.searchsorted(dst_s, d0)
            hi = np.searchsorted(dst_s, d1)
            s_rot = (src_s[lo:hi] - base) % n + 1     # rotated table row
            d_loc = dst_s[lo:hi] - d0                 # slot in block
            a_idx = dst_s[lo:hi] - base + 1           # rotated a_dst row
            is_lo = s_rot < lo_rows
            seg[(m, b)] = (s_rot, d_loc, a_idx, is_lo)
            tl_req[m, b] = -(-int(is_lo.sum()) // P)
            th_req[m, b] = -(-int((~is_lo).sum()) // P)
    tl = tl_req.max(axis=0).astype(int)
    th = th_req.max(axis=0).astype(int)
    for b in range(cfg.nblk):
        if tl[b] + th[b] == 0:
            tl[b] = 1

    # ---- tile column layout: group-major, half-major, block-major
    ngrp = -(-cfg.nblk // cfg.group)
    groups = [list(range(g * cfg.group, min((g + 1) * cfg.group, cfg.nblk)))
              for g in range(ngrp)]
    col0 = {}            # (b, half) -> first tile column
    blk_span = {}        # b -> (t_first, t_last) inclusive tile cols
    chunks = []          # dicts: b-range via tiles list, t0, nt, icol, half
    icol = 0
    T = 0
    for blks in groups:
        for half in ("lo", "hi"):
            cnt = tl if half == "lo" else th
            span_t0 = T
            for b in blks:
                col0[(b, half)] = T
                T += int(cnt[b])
            span_nt = T - span_t0
            # split the span into gather chunks of <= tc_max tiles
            q0 = 0
            while q0 < span_nt:
                qq = min(cfg.tc_max, span_nt - q0)
                assert qq * P <= cfg.ring, (qq, cfg.ring)
                chunks.append(dict(t0=span_t0 + q0, nt=qq, nidx=qq * P,
                                   icol=icol, half=half))
                icol += qq * P // 16
                q0 += qq
    icol_main = icol
    for b in range(cfg.nblk):
        ts, te = [], []
        for half, cnt in (("lo", tl), ("hi", th)):
            if cnt[b]:
                ts.append(col0[(b, half)])
                te.append(col0[(b, half)] + int(cnt[b]) - 1)
        blk_span[b] = (min(ts), max(te))

    # relT layout: partition b holds block b's rel values (lo tiles then hi
    # tiles, matching the global tile-column order restricted to b)
    assert cfg.nblk <= P
    rtc = int((tl + th).max()) * P

    per_core = []
    for m in range(cfg.n_cores):
        rel_all = np.zeros((P, T), dtype=np.float32)
        relt = np.zeros((P, rtc), dtype=np.float32)
        midx_all = np.zeros((P, icol_main), dtype=np.int16)
        sidx_all = np.zeros((P, T * P // 16), dtype=np.int16)  # per-tile src
        for b in range(cfg.nblk):
            s_rot, d_loc, a_idx, is_lo = seg[(m, b)]
            boff = 0
            for half, cnt in (("lo", tl), ("hi", th)):
                nt_half = int(cnt[b])
                if nt_half == 0:
                    continue
                sel = is_lo if half == "lo" else ~is_lo
                ne = int(sel.sum())
                npad = nt_half * P
                if half == "lo":
                    bs = np.zeros(npad, dtype=np.int64)       # sentinel_lo
                else:
                    bs = np.full(npad, sent_hi_rel + hi_base, dtype=np.int64)
                br = np.zeros(npad, dtype=np.int64)
                bs[:ne] = s_rot[sel]
                br[:ne] = d_loc[sel]
                if half == "hi":
                    bs -= hi_base
                assert bs.min() >= 0 and bs.max() < cfg.win
                t0 = col0[(b, half)]
                rel_all[:, t0:t0 + nt_half] = br.reshape(nt_half, P).T
                relt[b, boff:boff + npad] = br
                boff += npad
                sidx_all[:, t0 * P // 16:(t0 + nt_half) * P // 16] = \
                    _wrap16(bs)
        for ch in chunks:
            c0, c1 = ch["t0"] * P // 16, (ch["t0"] + ch["nt"]) * P // 16
            midx_all[:, ch["icol"]:ch["icol"] + ch["nidx"] // 16] = \
                sidx_all[:, c0:c1]
        rel2 = np.repeat(rel_all.astype(BF16NP), 2, axis=1)   # [P, 2T]
        per_core.append(dict(
            midx=np.ascontiguousarray(midx_all),
            relt=np.ascontiguousarray(relt.astype(BF16NP)),
            rel=np.ascontiguousarray(rel2),
        ))
    meta = dict(chunks=chunks, tl=tl, th=th, col0=col0, blk_span=blk_span,
                groups=groups, T=T, icol_main=icol_main, rtc=rtc,
                hi_base=hi_base)
    meta["off"] = _pack_offsets(cfg, meta)
    return per_core, meta


def _pack_offsets(cfg, meta):
    """Column offsets (int16 units) of each region in the packed input."""
    off = {}
    c = 0
    for name, cols in (
        ("midx", meta["icol_main"]),
        ("relt", meta["rtc"]),
        ("rel", 2 * meta["T"]),
        ("w", cfg.kt * cfg.rowp),
        ("bias", 2 * cfg.hc),
        ("pw", 2 * cfg.hc),
        ("sent", cfg.rowp),
        ("xt", cfg.kt * cfg.n_nodes),
    ):
        off[name] = c
        c += cols
    off["end"] = c
    return off


def pack_inputs(pc, shared, m, cfg, meta):
    """One [128, PKC] int16 tensor holding every per-core input."""
    off = meta["off"]
    pk = np.zeros((P, off["end"]), dtype=np.int16)

    def put(name, arr):
        a = np.ascontiguousarray(arr).view(np.int16)
        pk[:a.shape[0], off[name]:off[name] + a.shape[1]] = a

    put("midx", pc["midx"])
    put("relt", pc["relt"])
    put("rel", pc["rel"])
    # rhs_w pre-transposed to the matmul layout: w[p, k*rowp + r]
    rw = shared["rhs_w"]                      # [in_ch, rowp] bf16
    wt = rw.reshape(cfg.kt, P, cfg.rowp).transpose(1, 0, 2).reshape(
        P, cfg.kt * cfg.rowp)
    put("w", wt)
    put("bias", shared["bias_rep"])
    put("pw", shared["pw_rep"])
    put("sent", shared["sent"])
    # xT pre-rearranged to the xg layout: x[p, k*n + q] = xT[k*128+p, q]
    xr = shared["xTs"][m].reshape(cfg.kt, P, cfg.n_nodes).transpose(
        1, 0, 2).reshape(P, cfg.kt * cfg.n_nodes)
    put("xt", xr)
    return pk


def host_prep_weights(x, lin_w, att_src, att_dst, bias, prelu_w, cfg):
    n, ic, h, c = cfg.n_nodes, cfg.in_ch, cfg.heads, cfg.hid
    w3 = lin_w.astype(np.float64).reshape(h, c, ic)
    wa_src = (w3 * att_src.astype(np.float64).reshape(h, c, 1)).sum(1).T
    wa_dst = (w3 * att_dst.astype(np.float64).reshape(h, c, 1)).sum(1).T
    lwT = lin_w.astype(np.float64).T                           # [ic, 256]
    lwTi = lwT.reshape(ic, h, c).transpose(0, 2, 1).reshape(ic, h * c)
    rhs = np.concatenate([wa_dst, lwTi, wa_src], axis=1)       # [ic, 264]
    rhs_w = np.ascontiguousarray(rhs.astype(BF16NP))
    def inter(v):
        return v.reshape(h, c).T.reshape(h * c)
    bias_rep = np.ascontiguousarray(np.broadcast_to(
        inter(bias.astype(np.float32)), (P, cfg.hc)))
    pw_rep = np.ascontiguousarray(np.broadcast_to(
        inter(prelu_w.astype(np.float32)), (P, cfg.hc)))
    sent = np.zeros((2, cfg.rowp), dtype=BF16NP)
    sent[:, cfg.rowp - cfg.heads:] = SENT_NEG      # a_src cols
    xbf = x.astype(BF16NP)
    xTs = []
    for m in range(cfg.n_cores):
        r = np.roll(xbf, -m * cfg.ndst, axis=0)
        xTs.append(np.ascontiguousarray(r.T))
    return dict(rhs_w=rhs_w, bias_rep=bias_rep, pw_rep=pw_rep, sent=sent,
                xTs=xTs)


# ---------------------------------------------------------------- builder

def build(cfg, meta, bias_nonzero=True, parts=None):
    parts = parts if parts is not None else {
        "p1", "gather", "adst", "s1", "pcomp", "mm", "epi"}
    if "p1" in parts:
        parts |= {"p1x", "p1mm", "p1w"}
    n, row, hc, h = cfg.n_nodes, cfg.row, cfg.hc, cfg.heads
    nc = bacc.Bacc(dynamic_dma_scratch_size=cfg.dma_scratch,
                   num_swdge_queues=cfg.n_swq)

    off = meta["off"]
    pk = nc.declare_dram_parameter("pk", [P, off["end"]], I16,
                                   isOutput=False)
    out = nc.declare_dram_parameter("out", [cfg.ndst, hc], F32, isOutput=True)

    def pkv(name, cols, dt=None, rows=(0, P)):
        v = pk[rows[0]:rows[1], off[name]:off[name] + cols]
        return v.bitcast(dt) if dt is not None else v

    table = nc.dram_tensor("table", [cfg.rows, row], BF16)
    T = meta["T"]
    hi_base = meta["hi_base"]

    with tile.TileContext(nc) as tc:
        fpool_cm = tc.tile_pool(name="p2f", bufs=1)
        fpool = fpool_cm.__enter__()

        midx_sb = fpool.tile([P, meta["icol_main"]], I16)
        nc.sync.dma_start(out=midx_sb[:], in_=pkv("midx", meta["icol_main"]))
        rel_sb = fpool.tile([P, 2 * T], BF16)
        nc.sync.dma_start(out=rel_sb[:], in_=pkv("rel", 2 * T, BF16))
        bias_sb = fpool.tile([P, hc], F32)
        nc.sync.dma_start(out=bias_sb[:], in_=pkv("bias", 2 * hc, F32))
        pw_sb = fpool.tile([P, hc], F32)
        nc.sync.dma_start(out=pw_sb[:], in_=pkv("pw", 2 * hc, F32))

        iota_i = fpool.tile([P, P], mybir.dt.int32)
        nc.gpsimd.iota(iota_i[:], pattern=[[1, P]], base=0,
                       channel_multiplier=0)
        iota_f = fpool.tile([P, P], F32)
        nc.vector.tensor_copy(out=iota_f[:], in_=iota_i[:])
        iota_bf = fpool.tile([P, P], BF16)
        nc.vector.tensor_copy(out=iota_bf[:], in_=iota_f[:])
        # per-partition index column (for the transposed one-hot)
        iotap_i = fpool.tile([P, 1], mybir.dt.int32)
        nc.gpsimd.iota(iotap_i[:], pattern=[[0, 1]], base=0,
                       channel_multiplier=1)
        iotap_f = fpool.tile([P, 1], F32)
        nc.vector.tensor_copy(out=iotap_f[:], in_=iotap_i[:])
        # all-ones row for the PE partition-broadcast (ones^T @ rel_row)
        ones_row = fpool.tile([1, P], BF16)
        nc.vector.memset(ones_row[:], 1.0)
        # per-block a_dst of this core's own dst slots, filled in phase 1
        adst_own = fpool.tile([P, cfg.nblk, h], BF16)

        # ---------------- phase 1: build table ----------------
        with (
            tc.tile_pool(name="p1w", bufs=1) as wpool,
            tc.tile_pool(name="p1x", bufs=2) as xpool,
            tc.tile_pool(name="p1o", bufs=2) as opool,
            tc.tile_pool(name="p1ps", bufs=4, space="PSUM") as pspool,
        ):
            w_sb = wpool.tile([P, cfg.kt, cfg.rowp], BF16)
            nc.sync.dma_start(
                out=w_sb[:],
                in_=pkv("w", cfg.kt * cfg.rowp, BF16).rearrange(
                    "p (k r) -> p k r", k=cfg.kt))
            nc.sync.dma_start(out=table[0:1, 0:cfg.rowp],
                              in_=pkv("sent", cfg.rowp, BF16, rows=(0, 1)))
            nc.sync.dma_start(out=table[cfg.rows - 1:cfg.rows, 0:cfg.rowp],
                              in_=pkv("sent", cfg.rowp, BF16, rows=(1, 2)))

            nch = cfg.node_chunk
            p1_any = parts & {"p1x", "p1mm", "p1w"}
            p1_starts = list(range(0, n if p1_any else 0, nch))

            def p1_chunk(n0):
                nn = min(nch, n - n0)
                nt = -(-nn // P)
                xg = xpool.tile([P, cfg.kt, nch], BF16, tag="xg")
                if "p1x" in parts:
                    xtv = pkv("xt", cfg.kt * n, BF16).rearrange(
                        "p (k q) -> p k q", k=cfg.kt)
                    nc.sync.dma_start(
                        out=xg[:, :, :nn], in_=xtv[:, :, n0:n0 + nn])
                stg = opool.tile([P, nch // P, cfg.rowp], BF16, tag="stg")
                for ti in range(nt):
                    t0 = ti * P
                    mm = min(P, nn - t0)
                    if "p1mm" in parts:
                        ps = pspool.tile([P, cfg.rowp], F32, tag="ps")
                        for k in range(cfg.kt):
                            nc.tensor.matmul(
                                ps[:mm, :],
                                lhsT=xg[:, k, t0:t0 + mm],
                                rhs=w_sb[:, k, :],
                                start=(k == 0), stop=(k == cfg.kt - 1))
                        if ti % 3 == 2:
                            nc.vector.tensor_copy(out=stg[:mm, ti, :],
                                                  in_=ps[:mm, :])
                        else:
                            nc.scalar.copy(out=stg[:mm, ti, :],
                                           in_=ps[:mm, :])
                        # own-block a_dst (table rows 1+128b..) align with
                        # the psum tile grid: stash cols 0:4 for phase 2
                        gt = (n0 + t0) // P
                        if gt < cfg.nblk:
                            nc.vector.tensor_copy(
                                out=adst_own[:, gt, :], in_=ps[:, 0:h])
                if "p1w" in parts:
                    # one batched write: DRAM rows [1+n0, 1+n0+nn) cols
                    # 0:rowp at 768B row stride, via the ACT HWDGE queue.
                    # A partial last tile is written separately so garbage
                    # staging rows never land past the table.
                    ft, rem = divmod(nn, P)
                    if ft:
                        tv = table[1 + n0:1 + n0 + ft * P, 0:cfg.rowp]
                        dst = AP(tv.tensor, tv.offset,
                                 [[row, P], [P * row, ft], [1, cfg.rowp]])
                        nc.scalar.dma_start(out=dst, in_=stg[:, :ft, :])
                    if rem:
                        r0 = 1 + n0 + ft * P
                        nc.scalar.dma_start(
                            out=table[r0:r0 + rem, 0:cfg.rowp],
                            in_=stg[:rem, ft, :])

            for n0 in p1_starts:
                p1_chunk(n0)

        # barrier: all table rows written before the main gathers read them
        tc.strict_bb_all_engine_barrier()

        # ---------------- phase 2: edge pass ----------------
        with (
            tc.tile_pool(name="p2g", bufs=3) as gpool,
            tc.tile_pool(name="p2s", bufs=3) as s1pool,
            tc.tile_pool(name="p2e", bufs=3) as epool,
            tc.tile_pool(name="p2o", bufs=3) as obpool,
            tc.tile_pool(name="p2ps", bufs=2, space="PSUM") as ps2pool,
            tc.tile_pool(name="p2bc", bufs=2, space="PSUM") as bcpool,
            tc.tile_pool(name="p2ad", bufs=2, space="PSUM") as adpool,
            tc.tile_pool(name="p2rl", bufs=3) as rlpool,
        ):
            # chunks per group (tile ranges are group-contiguous)
            grp_chunks = [[] for _ in meta["groups"]]
            bounds = []
            t_acc = 0
            for gi, blks in enumerate(meta["groups"]):
                span = sum(int(meta["tl"][b] + meta["th"][b]) for b in blks)
                bounds.append((t_acc, t_acc + span))
                t_acc += span
            for ch in meta["chunks"]:
                for gi, (g0, g1) in enumerate(bounds):
                    if g0 <= ch["t0"] < g1:
                        grp_chunks[gi].append(ch)
                        break

            def do_epilogue(b, psb):
                den = epool.tile([P, h], F32, tag="den")
                nc.vector.tensor_scalar_add(out=den[:],
                                            in0=psb[:, hc:hc + h],
                                            scalar1=1e-6)
                rec = epool.tile([P, h], F32, tag="rec")
                nc.vector.reciprocal(out=rec[:], in_=den[:])
                ob = obpool.tile([P, hc], F32, tag="ob")
                recb = AP(rec.tensor, rec[:].offset,
                          [rec[:].ap[0], [0, cfg.hid], [1, h]])
                nc.vector.tensor_tensor(
                    out=ob[:].rearrange("p (c hh) -> p c hh", hh=h),
                    in0=psb[:, 0:hc].rearrange("p (c hh) -> p c hh", hh=h),
                    in1=recb, op=OP.mult)
                if bias_nonzero:
                    nc.vector.tensor_add(out=ob[:], in0=ob[:],
                                         in1=bias_sb[:])
                t2 = obpool.tile([P, hc], F32, tag="t2")
                nc.vector.scalar_tensor_tensor(
                    out=t2[:], in0=ob[:], scalar=0.0, op0=OP.min,
                    in1=pw_sb[:], op1=OP.mult)
                obp = obpool.tile([P, hc], F32, tag="obp")
                obpv = obp[:]
                # write through a permuted view: col c*4+hh -> hh*64+c
                obp_perm = AP(obpv.tensor, obpv.offset,
                              [obpv.ap[0], [cfg.hid, h], [1, cfg.hid]])
                iview = [[1, h], [h, cfg.hid]]
                ob_i = AP(ob[:].tensor, ob[:].offset, [ob[:].ap[0]] + iview)
                t2_i = AP(t2[:].tensor, t2[:].offset, [t2[:].ap[0]] + iview)
                nc.vector.scalar_tensor_tensor(
                    out=obp_perm, in0=ob_i, scalar=0.0, op0=OP.max,
                    in1=t2_i, op1=OP.add)
                rows = min(P, cfg.ndst - b * P)
                nc.sync.dma_start(out=out[b * P:b * P + rows, :],
                                  in_=obp[:rows, :])

            for gi, blks in enumerate(meta["groups"]):
                ps_of = {}
                for b in blks:
                    psb = ps2pool.tile([P, hc + h], F32, tag="psb")
                    ps_of[b] = psb
                first_t = {b: meta["blk_span"][b][0] for b in blks}
                last_t = {b: meta["blk_span"][b][1] for b in blks}
                rl_of = {}
                if "adst" in parts:
                    for b in blks:
                        ncols = int(meta["tl"][b] + meta["th"][b]) * P
                        rl = rlpool.tile([1, meta["rtc"]], BF16, tag="rl")
                        nc.sync.dma_start(
                            out=rl[0:1, 0:ncols],
                            in_=pkv("relt", meta["rtc"], BF16,
                                    rows=(b, b + 1))[:, 0:ncols])
                        rl_of[b] = rl
                for chi, ch in enumerate(grp_chunks[gi]):
                    qq = ch["nt"]
                    t0 = ch["t0"]
                    g = gpool.tile([P, qq, cfg.gcol], BF16, tag="g")
                    if "gather" in parts:
                        if ch["half"] == "lo":
                            in_ap = table[0:cfg.win, :]
                        else:
                            in_ap = table[hi_base:cfg.rows, :]
                        nc.gpsimd.dma_gather(
                            out_ap=g[:],
                            in_ap=in_ap,
                            idxs_ap=midx_sb[:, ch["icol"]:ch["icol"]
                                            + ch["nidx"] // 16],
                            num_idxs=ch["nidx"],
                            num_idxs_reg=ch["nidx"],
                            elem_size=cfg.gcol,
                            elem_step=row,
                            queue_num=(gi + chi) % cfg.n_swq)

                    # one-hot S1[k, q, slot] = (rel[k, q] == slot)
                    s1 = s1pool.tile([P, qq, P], BF16, tag="s1")
                    if "s1" in parts:
                        rsl = rel_sb[:, 2 * t0:2 * (t0 + qq)]
                        rel_b = AP(rsl.tensor, rsl.offset,
                                   [rsl.ap[0], [2, qq], [0, P // 2], [1, 2]])
                        iap = iota_bf[:]
                        iota_b = AP(iap.tensor, iap.offset,
                                    [iap.ap[0], [0, qq], [2, P // 2], [1, 2]])
                        s1v = s1[:]
                        s1_b = AP(s1v.tensor, s1v.offset,
                                  [s1v.ap[0], [P, qq], [2, P // 2], [1, 2]])
                        nc.vector.tensor_tensor(
                            out=s1_b, in0=rel_b, in1=iota_b, op=OP.is_equal)

                    if "adst" in parts:
                        # per-edge a_dst without any DMA:
                        # 1) broadcast rel (slot id per edge) to all
                        #    partitions via PE outer product ones^T@rel_row
                        # 2) transposed one-hot S1T[slot,e]=(slot==rel[e])
                        # 3) a_dst_edge[e,:] = S1T^T @ adst_own[:,b,:]
                        b = _block_of(meta, ch)
                        bl0 = _blk_local(meta, ch)
                        s1t = s1pool.tile([P, qq, P], BF16, tag="s1t")
                        for j0 in range(0, qq, 4):
                            jj = min(4, qq - j0)
                            bc = bcpool.tile([P, 4 * P], F32, tag="bc")
                            c0 = (bl0 + j0) * P
                            nc.tensor.matmul(
                                bc[:, 0:jj * P],
                                lhsT=ones_row[0:1, :],
                                rhs=rl_of[b][0:1, c0:c0 + jj * P],
                                start=True, stop=True)
                            ipb = AP(iotap_f.tensor, iotap_f[:].offset,
                                     [iotap_f[:].ap[0], [0, jj], [0, P]])
                            s1tv = s1t[:, j0:j0 + jj, :]
                            nc.vector.tensor_tensor(
                                out=s1tv, in0=ipb,
                                in1=bc[:, 0:jj * P].rearrange(
                                    "p (q e) -> p q e", q=jj),
                                op=OP.is_equal)
                        ps_ad = adpool.tile([P, qq, h], F32, tag="ps_ad")
                        for j in range(qq):
                            nc.tensor.matmul(
                                ps_ad[:, j, :],
                                lhsT=s1t[:, j, :],
                                rhs=adst_own[:, b, :],
                                start=True, stop=True)

                    if "pcomp" in parts:
                        # p = exp(leaky_relu(a_src + a_dst))
                        ef = epool.tile([P, qq, h], F32, tag="ef")
                        if "adst" in parts:
                            nc.vector.tensor_add(
                                out=ef[:],
                                in0=g[:, :, 4 + hc:4 + hc + h],
                                in1=ps_ad[:])
                        else:
                            nc.vector.tensor_copy(
                                out=ef[:], in_=g[:, :, 4 + hc:4 + hc + h])
                        # lrelu(e) = max(0.2*e, e) in one DVE op
                        nc.vector.scalar_tensor_tensor(
                            out=ef[:], in0=ef[:], scalar=NEG_SLOPE,
                            op0=OP.mult, in1=ef[:], op1=OP.max)
                        # exp on ACT, written straight into g as bf16
                        nc.scalar.activation(
                            out=g[:, :, 4 + hc:4 + hc + h], in_=ef[:],
                            func=AF.Exp)
                        # msg = h * p; h interleaved [c, hh] so the
                        # broadcast AP keeps a stride-1 last dim (2x DVE)
                        msg = g[:, :, 4:4 + hc].rearrange(
                            "p q (c hh) -> p q c hh", hh=h)
                        pslice = g[:, :, 4 + hc:4 + hc + h]
                        pb = AP(pslice.tensor, pslice.offset,
                                [pslice.ap[0], pslice.ap[1],
                                 [0, cfg.hid], [1, h]])
                        nc.vector.tensor_tensor(out=msg, in0=msg, in1=pb,
                                                op=OP.mult)

                    if "mm" in parts:
                        for j in range(qq):
                            tcol = t0 + j
                            b = next(bb for bb in blks
                                     if first_t[bb] <= tcol <= last_t[bb]
                                     and _tile_in_block(meta, bb, tcol))
                            nc.tensor.matmul(
                                ps_of[b][:, :],
                                lhsT=s1[:, j, :],
                                rhs=g[:, j, 4:4 + hc + h],
                                start=(tcol == first_t[b]),
                                stop=(tcol == last_t[b]))

                # epilogue: out = prelu(msg/denom + bias)
                if "epi" not in parts or "mm" not in parts:
                    continue
                for b in blks:
                    do_epilogue(b, ps_of[b][:])
        fpool_cm.__exit__(None, None, None)
    return nc


def _block_of(meta, ch):
    for (b, half), c0 in meta["col0"].items():
        if half != ch["half"]:
            continue
        cnt = meta["tl" if half == "lo" else "th"][b]
        if cnt and c0 <= ch["t0"] < c0 + int(cnt):
            assert ch["t0"] + ch["nt"] <= c0 + int(cnt), \
                "adst-mm path needs single-block chunks (cfg.group==1)"
            return b
    raise AssertionError(ch)


def _blk_local(meta, ch):
    b = _block_of(meta, ch)
    if ch["half"] == "lo":
        return ch["t0"] - meta["col0"][(b, "lo")]
    return int(meta["tl"][b]) + ch["t0"] - meta["col0"][(b, "hi")]


def _tile_in_block(meta, b, tcol):
    for half, cnt in (("lo", meta["tl"]), ("hi", meta["th"])):
        if cnt[b]:
            c0 = meta["col0"][(b, half)]
            if c0 <= tcol < c0 + int(cnt[b]):
                return True
    return False


# ---------------------------------------------------------------- runner

def _prepare(x, edge_index, lin_w, att_src, att_dst, bias, prelu_w, cfg,
             parts=None):
    per_core, meta = host_prep_edges(np.asarray(edge_index), cfg)
    shared = host_prep_weights(np.asarray(x), np.asarray(lin_w),
                               np.asarray(att_src), np.asarray(att_dst),
                               np.asarray(bias), np.asarray(prelu_w), cfg)
    bias_nonzero = bool(np.any(np.asarray(bias) != 0))
    nc = build(cfg, meta, bias_nonzero=bias_nonzero, parts=parts)
    in_maps = [dict(pk=pack_inputs(per_core[m], shared, m, cfg, meta))
               for m in range(cfg.n_cores)]
    return nc, in_maps


def _run_pjrt(nc, in_maps, n_cores, time_iters=0):
    """Mirror of bass2jax.run_bass_via_pjrt that keeps the compiled callable
    so warm re-executions can be timed (no NTFF profiling in this container).
    Returns (per-core result dicts, min warm wall ns or None)."""
    import time
    import jax
    from jax.sharding import Mesh, PartitionSpec
    from jax.experimental.shard_map import shard_map
    from concourse import bass2jax, mybir as mb

    bass2jax.install_neuronx_cc_hook()
    assert nc.dbg_addr is None
    partition_name = (nc.partition_id_tensor.name
                      if nc.partition_id_tensor else None)
    in_names, out_names, out_avals, zero_outs = [], [], [], []
    for alloc in nc.m.functions[0].allocations:
        if not isinstance(alloc, mb.MemoryLocationSet):
            continue
        name = alloc.memorylocations[0].name
        if alloc.kind == "ExternalInput":
            if name != partition_name:
                in_names.append(name)
        elif alloc.kind == "ExternalOutput":
            shape = tuple(alloc.tensor_shape)
            dtype = mb.dt.np(alloc.dtype)
            out_names.append(name)
            out_avals.append(jax.core.ShapedArray(shape, dtype))
            zero_outs.append(np.zeros(shape, dtype))
    n_params = len(in_names)
    in_names.extend(out_names)
    if partition_name is not None:
        in_names.append(partition_name)

    def _body(*args):
        operands = list(args)
        if partition_name is not None:
            operands.append(bass2jax.partition_id_tensor())
        outs = bass2jax._bass_exec_p.bind(
            *operands,
            out_avals=tuple(out_avals),
            in_names=tuple(in_names),
            out_names=tuple(out_names),
            lowering_input_output_aliases=(),
            sim_require_finite=True,
            sim_require_nnan=True,
            nc=nc,
        )
        return tuple(outs)

    devices = jax.devices()[:n_cores]
    mesh = Mesh(np.asarray(devices), ("core",))
    n_outs = len(out_avals)
    in_specs = (PartitionSpec("core"),) * (n_params + n_outs)
    out_specs = (PartitionSpec("core"),) * n_outs
    sharded = jax.jit(
        shard_map(_body, mesh=mesh, in_specs=in_specs, out_specs=out_specs,
                  check_rep=False),
        keep_unused=True,
    )
    per_core = [[np.asarray(m[name]) for name in in_names[:n_params]]
                for m in in_maps]
    concat_in = [
        np.concatenate([per_core[c][i] for c in range(n_cores)], axis=0)
        for i in range(n_params)
    ]
    concat_zeros = [
        np.zeros((n_cores * z.shape[0], *z.shape[1:]), z.dtype)
        for z in zero_outs
    ]
    sh = jax.sharding.NamedSharding(mesh, PartitionSpec("core"))
    dev_args = [jax.device_put(a, sh) for a in concat_in + concat_zeros]
    out_arrs = sharded(*dev_args)
    jax.block_until_ready(out_arrs)
    t_ns = None
    if time_iters > 0:
        # The axon RPC floor (~90 ms) swamps single-dispatch wall time, but
        # async dispatches pipeline on the device queue: time M back-to-back
        # executions blocking once, for two M values, and use the slope.
        def loop_wall(mreps):
            best = None
            for _ in range(time_iters):
                t0 = time.perf_counter_ns()
                o = None
                for _ in range(mreps):
                    o = sharded(*dev_args)
                jax.block_until_ready(o)
                dt = time.perf_counter_ns() - t0
                best = dt if best is None else min(best, dt)
            return best

        m1, m2 = 24, 72
        loop_wall(4)  # warm
        w1 = loop_wall(m1)
        w2 = loop_wall(m2)
        t_ns = max(0, (w2 - w1) // (m2 - m1))
    results = [
        {name: np.asarray(out_arrs[i]).reshape(n_cores, *out_avals[i].shape)[c]
         for i, name in enumerate(out_names)}
        for c in range(n_cores)
    ]
    return results, t_ns


def run(x, edge_index, lin_w, att_src, att_dst, bias, prelu_w,
        cfg=None, time_iters=0):
    cfg = cfg or CFG
    nc, in_maps = _prepare(x, edge_index, lin_w, att_src, att_dst, bias,
                           prelu_w, cfg)
    nc.finalize()
    results, t_ns = _run_pjrt(nc, in_maps, cfg.n_cores,
                              time_iters=time_iters)
    outs = [results[m]["out"] for m in range(cfg.n_cores)]
    full = np.concatenate(outs, axis=0).astype(np.float32)
    return full, t_ns


def kernel(**inputs):
    full, _ = run(inputs["x"], inputs["edge_index"], inputs["lin_w"],
                  inputs["att_src"], inputs["att_dst"], inputs["bias"],
                  inputs["prelu_w"])
    return full
